# revision 35
# baseline (speedup 1.0000x reference)
"""Grok1-style GQA attention (S=2048, H=6144, 48 Q heads / 8 KV heads, rope,
softcap-30, causal) as a Bass/Tile kernel sharded over 8 NeuronCores.

Sharding: tensor-parallel across heads. Core c owns Q heads 6c..6c+5 and KV
head c. Each core computes its qkv projection slice, rope, causal softcap
attention for its 6 Q heads against its single KV head, and a partial
o_proj (its 768 columns of w_o). The host sums the 8 partial outputs.

Numerics: softcap bounds scores to [-30, 30], so softmax is computed as
exp(30*tanh(s/30) - 30) with a *constant* bias — no running max.

v2 design (vs the naive per-block version):
 - The softmax denominator comes free from the PV matmul: V is augmented
   with a ones column (VN blocks are [k,129], col 128 = 1), and PV is done
   in [q, d] orientation (lhsT = probs [k,q], rhs = V_aug [k,129]) so the
   per-query denominator lands on the q PARTITION axis -> cheap per-partition
   reciprocal + scale on the vector engine. This kills the M=1 row-sum
   matmuls, the K=1 broadcast matmuls and the single-lane reciprocals.
 - tanh/exp run on [128, <=1024] batches spanning 2 PSUM banks (fewer ACT
   instructions, less fixed overhead).
 - Software pipelining by emission order: attention of chunk i is
   interleaved with the QKV projection of chunk i+1 (and attention of the
   last chunk with the first 3/4 of o_proj) so the tensor engine never
   stalls on the scalar engine and HAM stays warm.
 - Normalized attention outputs are transposed back to [d, q] in bulk at
   chunk end (PE transpose + DVE copy) for the o_proj lhsT.

Layouts (host-prepped, contraction dim on SBUF partitions):
  ht   [4,48,128,512] bf16  : ht[sc,hb,p,c] = hidden[sc*512+c, hb*128+p]
  wq   [8,128,48,128] bf16  : wq[ob,p,hb,o] = w_qkv_core[ob*128+o, hb*128+p]
  wo   [12,128,6,512] bf16  : wo[mc,p,fb,m] = (w_o[:,core]*MULT).T[fb*128+p, mc*512+m]
  cosf/sinf [128,2048] f32  : duplicated/sign-flipped rope tables (neox)
  triu [128,128] bf16       : triu[k,q] = 1 if q >= k else 0
"""

import sys
import numpy as np
from collections import deque

sys.path.insert(0, "/opt/trn_rl_repo")

import ml_dtypes

import concourse.bass as bass
import concourse.mybir as mybir
import concourse.tile as tile
from concourse import bacc
from concourse.bass_utils import run_bass_kernel_spmd

F32 = mybir.dt.float32
BF16 = mybir.dt.bfloat16
AF = mybir.ActivationFunctionType

S = 2048
HID = 6144
D = 128
NQ = 6          # q heads per core
N_CORES = 8
SCALE = D ** -0.5
SOFTCAP = 30.0
ATTN_MULT = 0.08838834764831845
ROPE_THETA = 10000.0

N_SC = 4        # s-chunks of 512
SCW = 512
N_HB = 48       # hidden 128-blocks
N_OB = 8        # output 128-blocks per core (6 Q | 1 K | 1 V)
N_MC = 12       # o_proj 512-col chunks
N_SB = 16       # s 128-blocks
N_FB = 6        # per-core o_proj feature 128-blocks (768/128)


def build_nc():
    nc = bacc.Bacc("TRN2", target_bir_lowering=False, debug=False, num_devices=N_CORES)

    ht_d = nc.dram_tensor("ht", [N_SC, 12, 128, 4 * SCW], BF16, kind="ExternalInput").ap()
    wq_d = nc.dram_tensor("wq", [N_OB, 128, N_HB, 128], BF16, kind="ExternalInput").ap()
    wo_d = nc.dram_tensor("wo", [N_MC, 128, N_FB, SCW], BF16, kind="ExternalInput").ap()
    cosf_d = nc.dram_tensor("cosf", [128, S], F32, kind="ExternalInput").ap()
    sinf_d = nc.dram_tensor("sinf", [128, S], F32, kind="ExternalInput").ap()
    triu_d = nc.dram_tensor("triu", [128, 128], BF16, kind="ExternalInput").ap()
    ident_d = nc.dram_tensor("ident", [128, 128], BF16, kind="ExternalInput").ap()
    negcap_d = nc.dram_tensor("negcap", [128, 1], F32, kind="ExternalInput").ap()
    out_d = nc.dram_tensor("out", [S, HID], BF16, kind="ExternalOutput").ap()

    from contextlib import ExitStack
    with tile.TileContext(nc) as tc, ExitStack() as ctx:
        const = ctx.enter_context(tc.tile_pool(name="const", bufs=1))
        pers = ctx.enter_context(tc.tile_pool(name="pers", bufs=1))
        htp = ctx.enter_context(tc.tile_pool(name="htp", bufs=12))
        wqp = ctx.enter_context(tc.tile_pool(name="wqp", bufs=2))
        wop = ctx.enter_context(tc.tile_pool(name="wop", bufs=2))
        ropep = ctx.enter_context(tc.tile_pool(name="ropep", bufs=4))
        stp = ctx.enter_context(tc.tile_pool(name="stp", bufs=2))
        ptp = ctx.enter_context(tc.tile_pool(name="ptp", bufs=8))
        nsp = ctx.enter_context(tc.tile_pool(name="nsp", bufs=6))
        rp = ctx.enter_context(tc.tile_pool(name="rp", bufs=4))
        otp = ctx.enter_context(tc.tile_pool(name="otp", bufs=4))
        ps_a = ctx.enter_context(tc.tile_pool(name="ps_a", bufs=2, space=bass.MemorySpace.PSUM))
        ps_sc = ctx.enter_context(tc.tile_pool(name="ps_sc", bufs=2, space=bass.MemorySpace.PSUM))
        ps_pv = ctx.enter_context(tc.tile_pool(name="ps_pv", bufs=2, space=bass.MemorySpace.PSUM))

        # ---------- persistent SBUF tiles (per s-chunk for precise deps) ----
        QT = [[pers.tile([128, SCW], BF16, tag=f"qt{h}_{c}", name=f"qt{h}_{c}")
               for c in range(N_SC)] for h in range(NQ)]
        KT = [pers.tile([128, SCW], BF16, tag=f"kt{c}", name=f"kt{c}") for c in range(N_SC)]
        VT = [pers.tile([128, SCW], BF16, tag=f"vt{c}", name=f"vt{c}") for c in range(N_SC)]
        VN = [pers.tile([128, 4 * 129], BF16, tag=f"vn{c}", name=f"vn{c}") for c in range(N_SC)]
        AOT = [[pers.tile([128, SCW], BF16, tag=f"aot{h}_{c}", name=f"aot{h}_{c}")
                for c in range(N_SC)] for h in range(NQ)]

        ht_tiles = {}

        def emit_ht_dma(sc, fine=False):
            lst = []
            for g in range(12):
                t = htp.tile([128, 4 * SCW], BF16, tag="ht", name="ht")
                if fine:
                    # chunk 0: split per-hb so the first matmul starts sooner
                    for g2 in range(4):
                        nc.sync.dma_start(t[:, g2 * SCW:(g2 + 1) * SCW],
                                          ht_d[sc, g][:, g2 * SCW:(g2 + 1) * SCW])
                else:
                    nc.sync.dma_start(t[:], ht_d[sc, g])
                lst.append(t)
            ht_tiles[sc] = lst

        # hidden chunk 0 first so the first matmul can start ASAP
        emit_ht_dma(0, fine=True)

        cosf = const.tile([128, S], F32, tag="cosf", name="cosf")
        sinf = const.tile([128, S], F32, tag="sinf", name="sinf")
        triu = const.tile([128, 128], BF16, tag="triu", name="triu")
        ident = const.tile([128, 128], BF16, tag="ident", name="ident")
        negcap = const.tile([128, 1], F32, tag="negcap", name="negcap")
        nc.sync.dma_start(triu[:], triu_d[:])
        nc.sync.dma_start(ident[:], ident_d[:])
        nc.sync.dma_start(negcap[:], negcap_d[:])

        # ---------------- QKV projection units (1 dma + 4 mm per ob) --------
        def make_qkv_units(sc):
            state = {}
            scs = slice(sc * SCW, (sc + 1) * SCW)

            def dma_unit(ob):
                w = wqp.tile([128, N_HB * 128], BF16, tag="wq", name="wq")
                for qd in range(4):
                    nc.gpsimd.dma_start(
                        w[:, qd * 1536:(qd + 1) * 1536],
                        wq_d[ob, :, qd * 12:(qd + 1) * 12])
                state[ob] = w

            def unit(ob, part):
                if part == 0:
                    ps = ps_a.tile([128, SCW], F32, tag="acc", name="acc")
                    state[(ob, "ps")] = ps
                w = state[ob]
                ps = state[(ob, "ps")]
                for hb in range(part * 12, part * 12 + 12):
                    nc.tensor.matmul(
                        ps[:], lhsT=w[:, hb * 128:(hb + 1) * 128],
                        rhs=ht_tiles[sc][hb // 4][:, (hb % 4) * SCW:(hb % 4 + 1) * SCW],
                        start=(hb == 0), stop=(hb == N_HB - 1))
                if part != 3:
                    return
                state.pop(ob)
                state.pop((ob, "ps"))
                if ob == 7:
                    nc.vector.tensor_copy(VT[sc][:], ps[:])
                    nc.vector.memset(VN[sc][:], 1.0)
                    tr = ps_sc.tile([128, 1024], BF16, tag="sc", name="sc")
                    for j in range(4):
                        nc.tensor.transpose(
                            tr[:, j * 128:(j + 1) * 128],
                            VT[sc][:, j * 128:(j + 1) * 128], ident[:])
                    for j in range(4):
                        nc.vector.tensor_copy(
                            VN[sc][:, j * 129:j * 129 + 128],
                            tr[:, j * 128:(j + 1) * 128])
                else:
                    rot = ropep.tile([128, SCW], F32, tag="rot", name="rot")
                    t1 = ropep.tile([128, SCW], F32, tag="t1", name="t1")
                    nc.scalar.copy(rot[0:64, :], ps[64:128, :])
                    nc.scalar.copy(rot[64:128, :], ps[0:64, :])
                    nc.vector.tensor_mul(t1[:], ps[:], cosf[:, scs])
                    nc.vector.tensor_mul(rot[:], rot[:], sinf[:, scs])
                    dst = QT[ob][sc] if ob < NQ else KT[sc]
                    nc.vector.tensor_add(dst[:], t1[:], rot[:])

            obs = (6, 7, 0, 1, 2, 3, 4, 5)        # K, V first, then Q heads
            units = [lambda: dma_unit(6), lambda: dma_unit(7)]
            for i, ob in enumerate(obs):
                for part in range(4):
                    units.append(lambda ob=ob, part=part: unit(ob, part))
                    if part == 0 and i + 2 < len(obs):
                        nxt = obs[i + 2]
                        units.append(lambda nxt=nxt: dma_unit(nxt))
            return units

        # ---------------- o_proj units --------------------------------------
        wo_state = {}

        def oproj_dma(mc, gen):
            def f():
                w = wop.tile([128, N_FB * SCW], BF16, tag="wo", name="wo")
                nc.gpsimd.dma_start(w[:], wo_d[mc])
                wo_state[(mc, gen)] = w
            return f

        def oproj_mm(mc, sb, gen, idx):
            def f():
                w = wo_state[(mc, gen)]
                if gen == 1 and idx % 2 == 0:
                    ps = ps_sc.tile([128, SCW], F32, tag="sc", name="sc")
                else:
                    ps = ps_a.tile([128, SCW], F32, tag="acc", name="acc")
                for fb in range(N_FB):
                    nc.tensor.matmul(
                        ps[:],
                        lhsT=AOT[fb][sb // 4][:, (sb % 4) * 128:(sb % 4) * 128 + 128],
                        rhs=w[:, fb * SCW:(fb + 1) * SCW],
                        start=(fb == 0), stop=(fb == N_FB - 1))
                ot = otp.tile([128, SCW], BF16, tag="ot", name="ot")
                if idx % 2 == 0:
                    nc.vector.tensor_copy(ot[:], ps[:])
                    nc.sync.dma_start(
                        out_d[sb * 128:(sb + 1) * 128, mc * SCW:(mc + 1) * SCW], ot[:])
                else:
                    nc.scalar.copy(ot[:], ps[:])
                    nc.scalar.dma_start(
                        out_d[sb * 128:(sb + 1) * 128, mc * SCW:(mc + 1) * SCW], ot[:])
            return f

        def make_oproj_units(sb_list, gen):
            units = [oproj_dma(0, gen), oproj_dma(1, gen)]
            idx = 0
            for mc in range(N_MC):
                for i, sb in enumerate(sb_list):
                    units.append(oproj_mm(mc, sb, gen, idx))
                    idx += 1
                    if i == 0 and mc + 2 < N_MC:
                        units.append(oproj_dma(mc + 2, gen))
            return units

        # ---------------- filler machinery ----------------------------------
        filler = deque()

        # ---------------- attention -----------------------------------------
        def batches_for(qc):
            bs = []
            for i in range(2 * qc):
                bs.append(dict(blocks=[(2 * i, 0, 512, 0), (2 * i + 1, 512, 512, 0)],
                               width=1024, diag=[]))
            base = 4 * qc
            bs.append(dict(blocks=[(base, 0, 512, 0), (base + 1, 512, 384, 128)],
                           width=896, diag=[0, 1]))
            bs.append(dict(blocks=[(base + 2, 0, 256, 256), (base + 3, 256, 128, 384)],
                           width=384, diag=[0, 1]))
            return bs

        def emit_attn(qc, hold=0):
            bs_proto = batches_for(qc)
            # per-drain-slot weights: attention PE-ns emitted before the slot
            weights = []
            for h in range(NQ):
                for b in bs_proto:
                    weights.append(sum(bl[2] for bl in b["blocks"]) * 0.43)
                for j in range(4):
                    weights.append((4 * qc + j + 1) * 81.0)
                weights.append(600.0)   # transpose group slot
            total_w = sum(weights)
            nfill0 = len(filler)
            st_drain = dict(done=0, si=0, cum=0.0)

            def drain():
                st_drain["cum"] += weights[st_drain["si"]]
                st_drain["si"] += 1
                target = min(round(nfill0 * st_drain["cum"] / total_w),
                             max(0, nfill0 - hold))
                while st_drain["done"] < target and filler:
                    filler.popleft()()
                    st_drain["done"] += 1

            for h in range(NQ):
                pt_map = {}
                for b in bs_proto:
                    sc_t = ps_sc.tile([128, 1024], F32, tag="sc", name="sc")
                    for (kb, off, w, q_lo) in b["blocks"]:
                        nc.tensor.matmul(
                            sc_t[:, off:off + w],
                            lhsT=KT[kb // 4][:, (kb % 4) * 128:(kb % 4) * 128 + 128],
                            rhs=QT[h][qc][:, q_lo:q_lo + w],
                            start=True, stop=True)
                    wdt = b["width"]
                    st = stp.tile([128, 1024], BF16, tag="st", name="st")
                    nc.scalar.activation(st[:, :wdt], sc_t[:, :wdt], AF.Tanh,
                                         scale=SCALE / SOFTCAP)
                    pt = ptp.tile([128, 1024], BF16, tag="pt", name="pt")
                    nc.scalar.activation(pt[:, :wdt], st[:, :wdt], AF.Exp,
                                         scale=SOFTCAP, bias=negcap[:])
                    for bi in b["diag"]:
                        (kb, off, w, q_lo) = b["blocks"][bi]
                        g = kb - 4 * qc
                        dcol = off + (g * 128 - q_lo)
                        nc.vector.tensor_mul(pt[:, dcol:dcol + 128],
                                             pt[:, dcol:dcol + 128], triu[:])
                    for (kb, off, w, q_lo) in b["blocks"]:
                        pt_map[kb] = (pt, off, q_lo)
                    drain()
                ns_h = []
                for j in range(4):
                    qb = 4 * qc + j
                    pv = ps_pv.tile([128, 129], F32, tag="pv", name="pv")
                    for kb in range(qb + 1):
                        pt, off, q_lo = pt_map[kb]
                        col = off + (j * 128 - q_lo)
                        nc.tensor.matmul(
                            pv[:],
                            lhsT=pt[:, col:col + 128],
                            rhs=VN[kb // 4][:, (kb % 4) * 129:(kb % 4) * 129 + 129],
                            start=(kb == 0), stop=(kb == qb))
                    r = rp.tile([128, 1], F32, tag="r", name="r")
                    nc.vector.reciprocal(r[:], pv[:, 128:129])
                    n = nsp.tile([128, 128], BF16, tag="ns", name="ns")
                    nc.vector.tensor_scalar_mul(n[:], pv[:, 0:128], r[:])
                    ns_h.append(n)
                    drain()
                drain()
                # transpose this head's normalized output back to [d, q]
                tr = ps_sc.tile([128, 512], BF16, tag="sc", name="sc")
                for j in range(4):
                    nc.tensor.transpose(tr[:, j * 128:(j + 1) * 128],
                                        ns_h[j][:], ident[:])
                for j in range(4):
                    nc.vector.tensor_copy(AOT[h][qc][:, j * 128:(j + 1) * 128],
                                          tr[:, j * 128:(j + 1) * 128])

        # ================= emission =========================================
        # preamble: K, V and Q-head-0 projections inline; the remaining
        # chunk-0 projection units become attention-0 filler
        pre = make_qkv_units(0)
        for i, u in enumerate(pre[:17]):
            u()
            if i == 1:
                # rope tables only needed from the first rope (~20us in);
                # deferring them keeps early DMA bandwidth for ht/wq
                nc.sync.dma_start(cosf[:], cosf_d[:])
                nc.sync.dma_start(sinf[:], sinf_d[:])
        filler.extend(pre[17:])
        for qc in range(N_SC):
            if qc + 1 < N_SC:
                emit_ht_dma(qc + 1)
                filler.extend(make_qkv_units(qc + 1))
            else:
                filler.extend(make_oproj_units(list(range(12)), gen=0))
            # hold a few filler units back across the boundary so the next
            # chunk's scalar-bound first head still has tensor work queued
            emit_attn(qc, hold=0 if qc == N_SC - 1 else 6)
            if qc == N_SC - 1:
                while filler:
                    filler.popleft()()
        for u in make_oproj_units([12, 13, 14, 15], gen=1):
            u()

    nc.compile()
    return nc


def prep_inputs(positions, hidden_states, w_qkv, w_o):
    """Host-side shard + relayout. Returns per-core input maps."""
    bf = ml_dtypes.bfloat16
    pos = np.asarray(positions).astype(np.float32)
    hidden = np.ascontiguousarray(np.asarray(hidden_states, dtype=np.float32))
    w_qkv = np.asarray(w_qkv, dtype=np.float32)
    w_o = np.asarray(w_o, dtype=np.float32)

    # rope tables (neox): freqs [S, 64]
    inv_freq = 1.0 / (ROPE_THETA ** (np.arange(0, D, 2, dtype=np.float32) / D))
    freqs = pos[:, None] * inv_freq[None, :]
    cos = np.cos(freqs).T.astype(np.float32)   # [64, S]
    sin = np.sin(freqs).T.astype(np.float32)
    cosf = np.concatenate([cos, cos], axis=0)               # [128, S]
    sinf = np.concatenate([-sin, sin], axis=0)

    triu = np.triu(np.ones((128, 128), np.float32)).astype(bf)  # [k, q]: q >= k
    ident = np.eye(128, dtype=np.float32).astype(bf)

    # ht[sc, hb, p, c] = hidden[sc*512+c, hb*128+p], regrouped 4 hb per tile:
    # htg[sc, g, p, g2*512+c] = ht[sc, 4g+g2, p, c]
    ht = np.ascontiguousarray(
        hidden.reshape(N_SC, SCW, N_HB, 128).transpose(0, 2, 3, 1)
        .reshape(N_SC, 12, 4, 128, SCW).transpose(0, 1, 3, 2, 4)
        .reshape(N_SC, 12, 128, 4 * SCW)).astype(bf)

    in_maps = []
    for c in range(N_CORES):
        q_rows = w_qkv[c * NQ * D:(c + 1) * NQ * D]          # [768, 6144]
        k_rows = w_qkv[HID + c * D:HID + (c + 1) * D]        # [128, 6144]
        v_rows = w_qkv[HID + 8 * D + c * D:HID + 8 * D + (c + 1) * D]
        wq_c = np.concatenate([q_rows, k_rows, v_rows], axis=0)  # [1024, 6144]
        # wq[ob, p, hb, o] = wq_c[ob*128+o, hb*128+p]
        wq_arr = np.ascontiguousarray(
            wq_c.reshape(N_OB, 128, N_HB, 128).transpose(0, 3, 2, 1)).astype(bf)
        wo_c = (w_o[:, c * NQ * D:(c + 1) * NQ * D] * ATTN_MULT).T  # [768, 6144]
        # wo[mc, p, fb, m] = wo_c[fb*128+p, mc*512+m]
        wo_arr = np.ascontiguousarray(
            wo_c.reshape(N_FB, 128, N_MC, SCW).transpose(2, 1, 0, 3)).astype(bf)
        in_maps.append({
            "ht": ht, "wq": wq_arr, "wo": wo_arr,
            "cosf": cosf, "sinf": sinf, "triu": triu, "ident": ident,
            "negcap": np.full((128, 1), -SOFTCAP, np.float32),
        })
    return in_maps


_NC_CACHE = None


def _get_nc():
    global _NC_CACHE
    if _NC_CACHE is None:
        _NC_CACHE = build_nc()
    return _NC_CACHE


def kernel(positions, hidden_states, w_qkv, w_o, _trace=False, _trace_kwargs=None):
    nc = _get_nc()
    in_maps = prep_inputs(positions, hidden_states, w_qkv, w_o)
    res = run_bass_kernel_spmd(nc, in_maps, list(range(N_CORES)),
                               trace=_trace, **(_trace_kwargs or {}))
    out = np.zeros((S, HID), np.float32)
    for c in range(N_CORES):
        out += res.results[c]["out"].astype(np.float32)
    out = out.astype(np.asarray(hidden_states).dtype)
    kernel.last_results = res
    return out


# revision 39
# speedup vs baseline: 1.0037x; 1.0037x over previous
"""Grok1-style GQA attention (S=2048, H=6144, 48 Q heads / 8 KV heads, rope,
softcap-30, causal) as a Bass/Tile kernel sharded over 8 NeuronCores.

Sharding: tensor-parallel across heads. Core c owns Q heads 6c..6c+5 and KV
head c. Each core computes its qkv projection slice, rope, causal softcap
attention for its 6 Q heads against its single KV head, and a partial
o_proj (its 768 columns of w_o). The host sums the 8 partial outputs.

Numerics: softcap bounds scores to [-30, 30], so softmax is computed as
exp(30*tanh(s/30) - 30) with a *constant* bias — no running max.

v2 design (vs the naive per-block version):
 - The softmax denominator comes free from the PV matmul: V is augmented
   with a ones column (VN blocks are [k,129], col 128 = 1), and PV is done
   in [q, d] orientation (lhsT = probs [k,q], rhs = V_aug [k,129]) so the
   per-query denominator lands on the q PARTITION axis -> cheap per-partition
   reciprocal + scale on the vector engine. This kills the M=1 row-sum
   matmuls, the K=1 broadcast matmuls and the single-lane reciprocals.
 - tanh/exp run on [128, <=1024] batches spanning 2 PSUM banks (fewer ACT
   instructions, less fixed overhead).
 - Software pipelining by emission order: attention of chunk i is
   interleaved with the QKV projection of chunk i+1 (and attention of the
   last chunk with the first 3/4 of o_proj) so the tensor engine never
   stalls on the scalar engine and HAM stays warm.
 - Normalized attention outputs are transposed back to [d, q] in bulk at
   chunk end (PE transpose + DVE copy) for the o_proj lhsT.

Layouts (host-prepped, contraction dim on SBUF partitions):
  ht   [4,48,128,512] bf16  : ht[sc,hb,p,c] = hidden[sc*512+c, hb*128+p]
  wq   [8,128,48,128] bf16  : wq[ob,p,hb,o] = w_qkv_core[ob*128+o, hb*128+p]
  wo   [12,128,6,512] bf16  : wo[mc,p,fb,m] = (w_o[:,core]*MULT).T[fb*128+p, mc*512+m]
  cosf/sinf [128,2048] f32  : duplicated/sign-flipped rope tables (neox)
  triu [128,128] bf16       : triu[k,q] = 1 if q >= k else 0
"""

import sys
import numpy as np
from collections import deque

sys.path.insert(0, "/opt/trn_rl_repo")

import ml_dtypes

import concourse.bass as bass
import concourse.mybir as mybir
import concourse.tile as tile
from concourse import bacc
from concourse.bass_utils import run_bass_kernel_spmd

F32 = mybir.dt.float32
BF16 = mybir.dt.bfloat16
AF = mybir.ActivationFunctionType

S = 2048
HID = 6144
D = 128
NQ = 6          # q heads per core
N_CORES = 8
SCALE = D ** -0.5
SOFTCAP = 30.0
ATTN_MULT = 0.08838834764831845
ROPE_THETA = 10000.0

N_SC = 4        # s-chunks of 512
SCW = 512
N_HB = 48       # hidden 128-blocks
N_OB = 8        # output 128-blocks per core (6 Q | 1 K | 1 V)
N_MC = 12       # o_proj 512-col chunks
N_SB = 16       # s 128-blocks
N_FB = 6        # per-core o_proj feature 128-blocks (768/128)


def build_nc():
    nc = bacc.Bacc("TRN2", target_bir_lowering=False, debug=False, num_devices=N_CORES)

    ht_d = nc.dram_tensor("ht", [N_SC, 12, 128, 4 * SCW], BF16, kind="ExternalInput").ap()
    wq_d = nc.dram_tensor("wq", [N_OB, 128, N_HB, 128], BF16, kind="ExternalInput").ap()
    wo_d = nc.dram_tensor("wo", [N_MC, 128, N_FB, SCW], BF16, kind="ExternalInput").ap()
    cosf_d = nc.dram_tensor("cosf", [128, S], F32, kind="ExternalInput").ap()
    sinf_d = nc.dram_tensor("sinf", [128, S], F32, kind="ExternalInput").ap()
    triu_d = nc.dram_tensor("triu", [128, 128], BF16, kind="ExternalInput").ap()
    ident_d = nc.dram_tensor("ident", [128, 128], BF16, kind="ExternalInput").ap()
    negcap_d = nc.dram_tensor("negcap", [128, 1], F32, kind="ExternalInput").ap()
    out_d = nc.dram_tensor("out", [S, HID], BF16, kind="ExternalOutput").ap()

    from contextlib import ExitStack
    with tile.TileContext(nc) as tc, ExitStack() as ctx:
        const = ctx.enter_context(tc.tile_pool(name="const", bufs=1))
        pers = ctx.enter_context(tc.tile_pool(name="pers", bufs=1))
        htp = ctx.enter_context(tc.tile_pool(name="htp", bufs=12))
        wqp = ctx.enter_context(tc.tile_pool(name="wqp", bufs=2))
        wop = ctx.enter_context(tc.tile_pool(name="wop", bufs=2))
        ropep = ctx.enter_context(tc.tile_pool(name="ropep", bufs=4))
        stp = ctx.enter_context(tc.tile_pool(name="stp", bufs=2))
        ptp = ctx.enter_context(tc.tile_pool(name="ptp", bufs=8))
        nsp = ctx.enter_context(tc.tile_pool(name="nsp", bufs=6))
        rp = ctx.enter_context(tc.tile_pool(name="rp", bufs=4))
        otp = ctx.enter_context(tc.tile_pool(name="otp", bufs=4))
        ps_a = ctx.enter_context(tc.tile_pool(name="ps_a", bufs=2, space=bass.MemorySpace.PSUM))
        ps_sc = ctx.enter_context(tc.tile_pool(name="ps_sc", bufs=2, space=bass.MemorySpace.PSUM))
        ps_pv = ctx.enter_context(tc.tile_pool(name="ps_pv", bufs=2, space=bass.MemorySpace.PSUM))

        # ---------- persistent SBUF tiles (per s-chunk for precise deps) ----
        QT = [[pers.tile([128, SCW], BF16, tag=f"qt{h}_{c}", name=f"qt{h}_{c}")
               for c in range(N_SC)] for h in range(NQ)]
        KT = [pers.tile([128, SCW], BF16, tag=f"kt{c}", name=f"kt{c}") for c in range(N_SC)]
        VT = [pers.tile([128, SCW], BF16, tag=f"vt{c}", name=f"vt{c}") for c in range(N_SC)]
        VN = [pers.tile([128, 4 * 129], BF16, tag=f"vn{c}", name=f"vn{c}") for c in range(N_SC)]
        AOT = [[pers.tile([128, SCW], BF16, tag=f"aot{h}_{c}", name=f"aot{h}_{c}")
                for c in range(N_SC)] for h in range(NQ)]

        ht_tiles = {}

        def emit_ht_dma(sc, fine=False):
            lst = []
            for g in range(12):
                t = htp.tile([128, 4 * SCW], BF16, tag="ht", name="ht")
                if fine:
                    # chunk 0: split per-hb so the first matmul starts sooner
                    for g2 in range(4):
                        nc.sync.dma_start(t[:, g2 * SCW:(g2 + 1) * SCW],
                                          ht_d[sc, g][:, g2 * SCW:(g2 + 1) * SCW])
                else:
                    nc.sync.dma_start(t[:], ht_d[sc, g])
                lst.append(t)
            ht_tiles[sc] = lst

        # hidden chunk 0 first so the first matmul can start ASAP
        emit_ht_dma(0, fine=True)

        cosf = const.tile([128, S], F32, tag="cosf", name="cosf")
        sinf = const.tile([128, S], F32, tag="sinf", name="sinf")
        triu = const.tile([128, 128], BF16, tag="triu", name="triu")
        ident = const.tile([128, 128], BF16, tag="ident", name="ident")
        negcap = const.tile([128, 1], F32, tag="negcap", name="negcap")
        nc.sync.dma_start(triu[:], triu_d[:])
        nc.sync.dma_start(ident[:], ident_d[:])
        nc.sync.dma_start(negcap[:], negcap_d[:])

        # ---------------- QKV projection units (4 per ob) -------------------
        def make_qkv_units(sc):
            state = {}
            scs = slice(sc * SCW, (sc + 1) * SCW)

            def unit(ob, part):
                if part == 0:
                    w = wqp.tile([128, N_HB * 128], BF16, tag="wq", name="wq")
                    ps = ps_a.tile([128, SCW], F32, tag="acc", name="acc")
                    state[ob] = (w, ps)
                    for qd in (0, 1):
                        nc.gpsimd.dma_start(
                            w[:, qd * 1536:(qd + 1) * 1536],
                            wq_d[ob, :, qd * 12:(qd + 1) * 12])
                w, ps = state[ob]
                if part == 2:
                    for qd in (2, 3):
                        nc.gpsimd.dma_start(
                            w[:, qd * 1536:(qd + 1) * 1536],
                            wq_d[ob, :, qd * 12:(qd + 1) * 12])
                for hb in range(part * 12, part * 12 + 12):
                    nc.tensor.matmul(
                        ps[:], lhsT=w[:, hb * 128:(hb + 1) * 128],
                        rhs=ht_tiles[sc][hb // 4][:, (hb % 4) * SCW:(hb % 4 + 1) * SCW],
                        start=(hb == 0), stop=(hb == N_HB - 1))
                if part != 3:
                    return
                state.pop(ob)
                if ob == 7:
                    nc.vector.tensor_copy(VT[sc][:], ps[:])
                    nc.vector.memset(VN[sc][:], 1.0)
                    tr = ps_sc.tile([128, 1024], BF16, tag="sc", name="sc")
                    for j in range(4):
                        nc.tensor.transpose(
                            tr[:, j * 128:(j + 1) * 128],
                            VT[sc][:, j * 128:(j + 1) * 128], ident[:])
                    for j in range(4):
                        nc.vector.tensor_copy(
                            VN[sc][:, j * 129:j * 129 + 128],
                            tr[:, j * 128:(j + 1) * 128])
                else:
                    rot = ropep.tile([128, SCW], F32, tag="rot", name="rot")
                    t1 = ropep.tile([128, SCW], F32, tag="t1", name="t1")
                    nc.scalar.copy(rot[0:64, :], ps[64:128, :])
                    nc.scalar.copy(rot[64:128, :], ps[0:64, :])
                    nc.vector.tensor_mul(t1[:], ps[:], cosf[:, scs])
                    nc.vector.tensor_mul(rot[:], rot[:], sinf[:, scs])
                    dst = QT[ob][sc] if ob < NQ else KT[sc]
                    nc.vector.tensor_add(dst[:], t1[:], rot[:])

            units = []
            for ob in (6, 7, 0, 1, 2, 3, 4, 5):   # K, V first, then Q heads
                for part in range(4):
                    units.append(lambda ob=ob, part=part: unit(ob, part))
            return units

        # ---------------- o_proj units --------------------------------------
        wo_state = {}

        def oproj_dma(mc, gen):
            def f():
                w = wop.tile([128, N_FB * SCW], BF16, tag="wo", name="wo")
                nc.gpsimd.dma_start(w[:], wo_d[mc])
                wo_state[(mc, gen)] = w
            return f

        def oproj_mm(mc, sb, gen, idx):
            def f():
                w = wo_state[(mc, gen)]
                if gen == 1 and idx % 2 == 0:
                    ps = ps_sc.tile([128, SCW], F32, tag="sc", name="sc")
                else:
                    ps = ps_a.tile([128, SCW], F32, tag="acc", name="acc")
                for fb in range(N_FB):
                    nc.tensor.matmul(
                        ps[:],
                        lhsT=AOT[fb][sb // 4][:, (sb % 4) * 128:(sb % 4) * 128 + 128],
                        rhs=w[:, fb * SCW:(fb + 1) * SCW],
                        start=(fb == 0), stop=(fb == N_FB - 1))
                ot = otp.tile([128, SCW], BF16, tag="ot", name="ot")
                if idx % 2 == 0:
                    nc.vector.tensor_copy(ot[:], ps[:])
                    nc.sync.dma_start(
                        out_d[sb * 128:(sb + 1) * 128, mc * SCW:(mc + 1) * SCW], ot[:])
                else:
                    nc.scalar.copy(ot[:], ps[:])
                    nc.scalar.dma_start(
                        out_d[sb * 128:(sb + 1) * 128, mc * SCW:(mc + 1) * SCW], ot[:])
            return f

        def make_oproj_units(sb_list, gen):
            units = [oproj_dma(0, gen), oproj_dma(1, gen)]
            idx = 0
            for mc in range(N_MC):
                for i, sb in enumerate(sb_list):
                    units.append(oproj_mm(mc, sb, gen, idx))
                    idx += 1
                    if i == 0 and mc + 2 < N_MC:
                        units.append(oproj_dma(mc + 2, gen))
            return units

        # ---------------- filler machinery ----------------------------------
        filler = deque()

        # ---------------- attention -----------------------------------------
        def batches_for(qc):
            bs = []
            for i in range(2 * qc):
                bs.append(dict(blocks=[(2 * i, 0, 512, 0), (2 * i + 1, 512, 512, 0)],
                               width=1024, diag=[]))
            base = 4 * qc
            bs.append(dict(blocks=[(base, 0, 512, 0), (base + 1, 512, 384, 128)],
                           width=896, diag=[0, 1]))
            bs.append(dict(blocks=[(base + 2, 0, 256, 256), (base + 3, 256, 128, 384)],
                           width=384, diag=[0, 1]))
            return bs

        def emit_attn(qc, hold=0):
            bs_proto = batches_for(qc)
            # per-drain-slot weights: attention PE-ns emitted before the slot
            weights = []
            for h in range(NQ):
                for b in bs_proto:
                    weights.append(sum(bl[2] for bl in b["blocks"]) * 0.43)
                for j in range(4):
                    weights.append((4 * qc + j + 1) * 81.0)
                weights.append(600.0)   # transpose group slot
            total_w = sum(weights)
            nfill0 = len(filler)
            st_drain = dict(done=0, si=0, cum=0.0)

            def drain():
                st_drain["cum"] += weights[st_drain["si"]]
                st_drain["si"] += 1
                target = min(round(nfill0 * st_drain["cum"] / total_w),
                             max(0, nfill0 - hold))
                while st_drain["done"] < target and filler:
                    filler.popleft()()
                    st_drain["done"] += 1

            for h in range(NQ):
                pt_map = {}
                for b in bs_proto:
                    sc_t = ps_sc.tile([128, 1024], F32, tag="sc", name="sc")
                    for (kb, off, w, q_lo) in b["blocks"]:
                        nc.tensor.matmul(
                            sc_t[:, off:off + w],
                            lhsT=KT[kb // 4][:, (kb % 4) * 128:(kb % 4) * 128 + 128],
                            rhs=QT[h][qc][:, q_lo:q_lo + w],
                            start=True, stop=True)
                    wdt = b["width"]
                    st = stp.tile([128, 1024], BF16, tag="st", name="st")
                    nc.scalar.activation(st[:, :wdt], sc_t[:, :wdt], AF.Tanh,
                                         scale=SCALE / SOFTCAP)
                    pt = ptp.tile([128, 1024], BF16, tag="pt", name="pt")
                    nc.scalar.activation(pt[:, :wdt], st[:, :wdt], AF.Exp,
                                         scale=SOFTCAP, bias=negcap[:])
                    for bi in b["diag"]:
                        (kb, off, w, q_lo) = b["blocks"][bi]
                        g = kb - 4 * qc
                        dcol = off + (g * 128 - q_lo)
                        nc.vector.tensor_mul(pt[:, dcol:dcol + 128],
                                             pt[:, dcol:dcol + 128], triu[:])
                    for (kb, off, w, q_lo) in b["blocks"]:
                        pt_map[kb] = (pt, off, q_lo)
                    drain()
                ns_h = []
                for j in range(4):
                    qb = 4 * qc + j
                    pv = ps_pv.tile([128, 129], F32, tag="pv", name="pv")
                    for kb in range(qb + 1):
                        pt, off, q_lo = pt_map[kb]
                        col = off + (j * 128 - q_lo)
                        nc.tensor.matmul(
                            pv[:],
                            lhsT=pt[:, col:col + 128],
                            rhs=VN[kb // 4][:, (kb % 4) * 129:(kb % 4) * 129 + 129],
                            start=(kb == 0), stop=(kb == qb))
                    r = rp.tile([128, 1], F32, tag="r", name="r")
                    nc.vector.reciprocal(r[:], pv[:, 128:129])
                    n = nsp.tile([128, 128], BF16, tag="ns", name="ns")
                    nc.vector.tensor_scalar_mul(n[:], pv[:, 0:128], r[:])
                    ns_h.append(n)
                    drain()
                drain()
                # transpose this head's normalized output back to [d, q]
                tr = ps_sc.tile([128, 512], BF16, tag="sc", name="sc")
                for j in range(4):
                    nc.tensor.transpose(tr[:, j * 128:(j + 1) * 128],
                                        ns_h[j][:], ident[:])
                for j in range(4):
                    nc.vector.tensor_copy(AOT[h][qc][:, j * 128:(j + 1) * 128],
                                          tr[:, j * 128:(j + 1) * 128])

        # ================= emission =========================================
        # preamble: K, V and Q-head-0 projections inline; the remaining
        # chunk-0 projection units become attention-0 filler
        pre = make_qkv_units(0)
        for i, u in enumerate(pre[:12]):
            u()
            if i == 1:
                # rope tables only needed from the first rope (~20us in);
                # deferring them keeps early DMA bandwidth for ht/wq
                nc.sync.dma_start(cosf[:], cosf_d[:])
                nc.sync.dma_start(sinf[:], sinf_d[:])
        filler.extend(pre[12:])
        for qc in range(N_SC):
            if qc + 1 < N_SC:
                emit_ht_dma(qc + 1)
                filler.extend(make_qkv_units(qc + 1))
            else:
                filler.extend(make_oproj_units(list(range(12)), gen=0))
            # hold a few filler units back across the boundary so the next
            # chunk's scalar-bound first head still has tensor work queued
            emit_attn(qc, hold=0 if qc == N_SC - 1 else 6)
            if qc == N_SC - 1:
                while filler:
                    filler.popleft()()
        for u in make_oproj_units([12, 13, 14, 15], gen=1):
            u()

    nc.compile()
    return nc


def prep_inputs(positions, hidden_states, w_qkv, w_o):
    """Host-side shard + relayout. Returns per-core input maps."""
    bf = ml_dtypes.bfloat16
    pos = np.asarray(positions).astype(np.float32)
    hidden = np.ascontiguousarray(np.asarray(hidden_states, dtype=np.float32))
    w_qkv = np.asarray(w_qkv, dtype=np.float32)
    w_o = np.asarray(w_o, dtype=np.float32)

    # rope tables (neox): freqs [S, 64]
    inv_freq = 1.0 / (ROPE_THETA ** (np.arange(0, D, 2, dtype=np.float32) / D))
    freqs = pos[:, None] * inv_freq[None, :]
    cos = np.cos(freqs).T.astype(np.float32)   # [64, S]
    sin = np.sin(freqs).T.astype(np.float32)
    cosf = np.concatenate([cos, cos], axis=0)               # [128, S]
    sinf = np.concatenate([-sin, sin], axis=0)

    triu = np.triu(np.ones((128, 128), np.float32)).astype(bf)  # [k, q]: q >= k
    ident = np.eye(128, dtype=np.float32).astype(bf)

    # ht[sc, hb, p, c] = hidden[sc*512+c, hb*128+p], regrouped 4 hb per tile:
    # htg[sc, g, p, g2*512+c] = ht[sc, 4g+g2, p, c]
    ht = np.ascontiguousarray(
        hidden.reshape(N_SC, SCW, N_HB, 128).transpose(0, 2, 3, 1)
        .reshape(N_SC, 12, 4, 128, SCW).transpose(0, 1, 3, 2, 4)
        .reshape(N_SC, 12, 128, 4 * SCW)).astype(bf)

    in_maps = []
    for c in range(N_CORES):
        q_rows = w_qkv[c * NQ * D:(c + 1) * NQ * D]          # [768, 6144]
        k_rows = w_qkv[HID + c * D:HID + (c + 1) * D]        # [128, 6144]
        v_rows = w_qkv[HID + 8 * D + c * D:HID + 8 * D + (c + 1) * D]
        wq_c = np.concatenate([q_rows, k_rows, v_rows], axis=0)  # [1024, 6144]
        # wq[ob, p, hb, o] = wq_c[ob*128+o, hb*128+p]
        wq_arr = np.ascontiguousarray(
            wq_c.reshape(N_OB, 128, N_HB, 128).transpose(0, 3, 2, 1)).astype(bf)
        wo_c = (w_o[:, c * NQ * D:(c + 1) * NQ * D] * ATTN_MULT).T  # [768, 6144]
        # wo[mc, p, fb, m] = wo_c[fb*128+p, mc*512+m]
        wo_arr = np.ascontiguousarray(
            wo_c.reshape(N_FB, 128, N_MC, SCW).transpose(2, 1, 0, 3)).astype(bf)
        in_maps.append({
            "ht": ht, "wq": wq_arr, "wo": wo_arr,
            "cosf": cosf, "sinf": sinf, "triu": triu, "ident": ident,
            "negcap": np.full((128, 1), -SOFTCAP, np.float32),
        })
    return in_maps


_NC_CACHE = None


def _get_nc():
    global _NC_CACHE
    if _NC_CACHE is None:
        _NC_CACHE = build_nc()
    return _NC_CACHE


def kernel(positions, hidden_states, w_qkv, w_o, _trace=False, _trace_kwargs=None):
    nc = _get_nc()
    in_maps = prep_inputs(positions, hidden_states, w_qkv, w_o)
    res = run_bass_kernel_spmd(nc, in_maps, list(range(N_CORES)),
                               trace=_trace, **(_trace_kwargs or {}))
    out = np.zeros((S, HID), np.float32)
    for c in range(N_CORES):
        out += res.results[c]["out"].astype(np.float32)
    out = out.astype(np.asarray(hidden_states).dtype)
    kernel.last_results = res
    return out


# revision 40
# speedup vs baseline: 1.0083x; 1.0046x over previous
"""Grok1-style GQA attention (S=2048, H=6144, 48 Q heads / 8 KV heads, rope,
softcap-30, causal) as a Bass/Tile kernel sharded over 8 NeuronCores.

Sharding: tensor-parallel across heads. Core c owns Q heads 6c..6c+5 and KV
head c. Each core computes its qkv projection slice, rope, causal softcap
attention for its 6 Q heads against its single KV head, and a partial
o_proj (its 768 columns of w_o). The host sums the 8 partial outputs.

Numerics: softcap bounds scores to [-30, 30], so softmax is computed as
exp(30*tanh(s/30) - 30) with a *constant* bias — no running max.

v2 design (vs the naive per-block version):
 - The softmax denominator comes free from the PV matmul: V is augmented
   with a ones column (VN blocks are [k,129], col 128 = 1), and PV is done
   in [q, d] orientation (lhsT = probs [k,q], rhs = V_aug [k,129]) so the
   per-query denominator lands on the q PARTITION axis -> cheap per-partition
   reciprocal + scale on the vector engine. This kills the M=1 row-sum
   matmuls, the K=1 broadcast matmuls and the single-lane reciprocals.
 - tanh/exp run on [128, <=1024] batches spanning 2 PSUM banks (fewer ACT
   instructions, less fixed overhead).
 - Software pipelining by emission order: attention of chunk i is
   interleaved with the QKV projection of chunk i+1 (and attention of the
   last chunk with the first 3/4 of o_proj) so the tensor engine never
   stalls on the scalar engine and HAM stays warm.
 - Normalized attention outputs are transposed back to [d, q] in bulk at
   chunk end (PE transpose + DVE copy) for the o_proj lhsT.

Layouts (host-prepped, contraction dim on SBUF partitions):
  ht   [4,48,128,512] bf16  : ht[sc,hb,p,c] = hidden[sc*512+c, hb*128+p]
  wq   [8,128,48,128] bf16  : wq[ob,p,hb,o] = w_qkv_core[ob*128+o, hb*128+p]
  wo   [12,128,6,512] bf16  : wo[mc,p,fb,m] = (w_o[:,core]*MULT).T[fb*128+p, mc*512+m]
  cosf/sinf [128,2048] f32  : duplicated/sign-flipped rope tables (neox)
  triu [128,128] bf16       : triu[k,q] = 1 if q >= k else 0
"""

import sys
import numpy as np
from collections import deque

sys.path.insert(0, "/opt/trn_rl_repo")

import ml_dtypes

import concourse.bass as bass
import concourse.mybir as mybir
import concourse.tile as tile
from concourse import bacc
from concourse.bass_utils import run_bass_kernel_spmd

F32 = mybir.dt.float32
BF16 = mybir.dt.bfloat16
AF = mybir.ActivationFunctionType

S = 2048
HID = 6144
D = 128
NQ = 6          # q heads per core
N_CORES = 8
SCALE = D ** -0.5
SOFTCAP = 30.0
ATTN_MULT = 0.08838834764831845
ROPE_THETA = 10000.0

N_SC = 4        # s-chunks of 512
SCW = 512
N_HB = 48       # hidden 128-blocks
N_OB = 8        # output 128-blocks per core (6 Q | 1 K | 1 V)
N_MC = 12       # o_proj 512-col chunks
N_SB = 16       # s 128-blocks
N_FB = 6        # per-core o_proj feature 128-blocks (768/128)


def build_nc():
    nc = bacc.Bacc("TRN2", target_bir_lowering=False, debug=False, num_devices=N_CORES)

    ht_d = nc.dram_tensor("ht", [N_SC, 12, 128, 4 * SCW], BF16, kind="ExternalInput").ap()
    wq_d = nc.dram_tensor("wq", [N_OB, 128, N_HB, 128], BF16, kind="ExternalInput").ap()
    wo_d = nc.dram_tensor("wo", [N_MC, 128, N_FB, SCW], BF16, kind="ExternalInput").ap()
    cosf_d = nc.dram_tensor("cosf", [128, S], F32, kind="ExternalInput").ap()
    sinf_d = nc.dram_tensor("sinf", [128, S], F32, kind="ExternalInput").ap()
    triu_d = nc.dram_tensor("triu", [128, 128], BF16, kind="ExternalInput").ap()
    ident_d = nc.dram_tensor("ident", [128, 128], BF16, kind="ExternalInput").ap()
    negcap_d = nc.dram_tensor("negcap", [128, 1], F32, kind="ExternalInput").ap()
    out_d = nc.dram_tensor("out", [S, HID], BF16, kind="ExternalOutput").ap()

    from contextlib import ExitStack
    with tile.TileContext(nc) as tc, ExitStack() as ctx:
        const = ctx.enter_context(tc.tile_pool(name="const", bufs=1))
        pers = ctx.enter_context(tc.tile_pool(name="pers", bufs=1))
        htp = ctx.enter_context(tc.tile_pool(name="htp", bufs=12))
        wqp = ctx.enter_context(tc.tile_pool(name="wqp", bufs=2))
        wop = ctx.enter_context(tc.tile_pool(name="wop", bufs=2))
        ropep = ctx.enter_context(tc.tile_pool(name="ropep", bufs=4))
        stp = ctx.enter_context(tc.tile_pool(name="stp", bufs=2))
        ptp = ctx.enter_context(tc.tile_pool(name="ptp", bufs=8))
        nsp = ctx.enter_context(tc.tile_pool(name="nsp", bufs=6))
        rp = ctx.enter_context(tc.tile_pool(name="rp", bufs=4))
        otp = ctx.enter_context(tc.tile_pool(name="otp", bufs=4))
        ps_a = ctx.enter_context(tc.tile_pool(name="ps_a", bufs=2, space=bass.MemorySpace.PSUM))
        ps_sc = ctx.enter_context(tc.tile_pool(name="ps_sc", bufs=2, space=bass.MemorySpace.PSUM))
        ps_pv = ctx.enter_context(tc.tile_pool(name="ps_pv", bufs=2, space=bass.MemorySpace.PSUM))

        # ---------- persistent SBUF tiles (per s-chunk for precise deps) ----
        QT = [[pers.tile([128, SCW], BF16, tag=f"qt{h}_{c}", name=f"qt{h}_{c}")
               for c in range(N_SC)] for h in range(NQ)]
        KT = [pers.tile([128, SCW], BF16, tag=f"kt{c}", name=f"kt{c}") for c in range(N_SC)]
        VT = [pers.tile([128, SCW], BF16, tag=f"vt{c}", name=f"vt{c}") for c in range(N_SC)]
        VN = [pers.tile([128, 4 * 129], BF16, tag=f"vn{c}", name=f"vn{c}") for c in range(N_SC)]
        AOT = [[pers.tile([128, SCW], BF16, tag=f"aot{h}_{c}", name=f"aot{h}_{c}")
                for c in range(N_SC)] for h in range(NQ)]

        ht_tiles = {}

        def emit_ht_dma(sc, fine=False):
            lst = []
            for g in range(12):
                t = htp.tile([128, 4 * SCW], BF16, tag="ht", name="ht")
                if fine:
                    # chunk 0: split per-hb so the first matmul starts sooner
                    for g2 in range(4):
                        nc.sync.dma_start(t[:, g2 * SCW:(g2 + 1) * SCW],
                                          ht_d[sc, g][:, g2 * SCW:(g2 + 1) * SCW])
                else:
                    nc.sync.dma_start(t[:], ht_d[sc, g])
                lst.append(t)
            ht_tiles[sc] = lst

        # hidden chunk 0 first so the first matmul can start ASAP
        emit_ht_dma(0)

        cosf = const.tile([128, S], F32, tag="cosf", name="cosf")
        sinf = const.tile([128, S], F32, tag="sinf", name="sinf")
        triu = const.tile([128, 128], BF16, tag="triu", name="triu")
        ident = const.tile([128, 128], BF16, tag="ident", name="ident")
        negcap = const.tile([128, 1], F32, tag="negcap", name="negcap")
        nc.sync.dma_start(triu[:], triu_d[:])
        nc.sync.dma_start(ident[:], ident_d[:])
        nc.sync.dma_start(negcap[:], negcap_d[:])

        # ---------------- QKV projection units (4 per ob) -------------------
        def make_qkv_units(sc):
            state = {}
            scs = slice(sc * SCW, (sc + 1) * SCW)

            def unit(ob, part):
                if part == 0:
                    w = wqp.tile([128, N_HB * 128], BF16, tag="wq", name="wq")
                    ps = ps_a.tile([128, SCW], F32, tag="acc", name="acc")
                    state[ob] = (w, ps)
                    for qd in (0, 1):
                        nc.gpsimd.dma_start(
                            w[:, qd * 1536:(qd + 1) * 1536],
                            wq_d[ob, :, qd * 12:(qd + 1) * 12])
                w, ps = state[ob]
                if part == 2:
                    for qd in (2, 3):
                        nc.gpsimd.dma_start(
                            w[:, qd * 1536:(qd + 1) * 1536],
                            wq_d[ob, :, qd * 12:(qd + 1) * 12])
                for hb in range(part * 12, part * 12 + 12):
                    nc.tensor.matmul(
                        ps[:], lhsT=w[:, hb * 128:(hb + 1) * 128],
                        rhs=ht_tiles[sc][hb // 4][:, (hb % 4) * SCW:(hb % 4 + 1) * SCW],
                        start=(hb == 0), stop=(hb == N_HB - 1))
                if part != 3:
                    return
                state.pop(ob)
                if ob == 7:
                    nc.vector.tensor_copy(VT[sc][:], ps[:])
                    nc.vector.memset(VN[sc][:], 1.0)
                    tr = ps_sc.tile([128, 1024], BF16, tag="sc", name="sc")
                    for j in range(4):
                        nc.tensor.transpose(
                            tr[:, j * 128:(j + 1) * 128],
                            VT[sc][:, j * 128:(j + 1) * 128], ident[:])
                    for j in range(4):
                        nc.vector.tensor_copy(
                            VN[sc][:, j * 129:j * 129 + 128],
                            tr[:, j * 128:(j + 1) * 128])
                else:
                    rot = ropep.tile([128, SCW], F32, tag="rot", name="rot")
                    t1 = ropep.tile([128, SCW], F32, tag="t1", name="t1")
                    nc.scalar.copy(rot[0:64, :], ps[64:128, :])
                    nc.scalar.copy(rot[64:128, :], ps[0:64, :])
                    nc.vector.tensor_mul(t1[:], ps[:], cosf[:, scs])
                    nc.vector.tensor_mul(rot[:], rot[:], sinf[:, scs])
                    dst = QT[ob][sc] if ob < NQ else KT[sc]
                    nc.vector.tensor_add(dst[:], t1[:], rot[:])

            units = []
            for ob in (6, 7, 0, 1, 2, 3, 4, 5):   # K, V first, then Q heads
                for part in range(4):
                    units.append(lambda ob=ob, part=part: unit(ob, part))
            return units

        # ---------------- o_proj units --------------------------------------
        wo_state = {}

        def oproj_dma(mc, gen):
            def f():
                w = wop.tile([128, N_FB * SCW], BF16, tag="wo", name="wo")
                nc.gpsimd.dma_start(w[:], wo_d[mc])
                wo_state[(mc, gen)] = w
            return f

        def oproj_mm(mc, sb, gen, idx):
            def f():
                w = wo_state[(mc, gen)]
                if gen == 1 and idx % 2 == 0:
                    ps = ps_sc.tile([128, SCW], F32, tag="sc", name="sc")
                else:
                    ps = ps_a.tile([128, SCW], F32, tag="acc", name="acc")
                for fb in range(N_FB):
                    nc.tensor.matmul(
                        ps[:],
                        lhsT=AOT[fb][sb // 4][:, (sb % 4) * 128:(sb % 4) * 128 + 128],
                        rhs=w[:, fb * SCW:(fb + 1) * SCW],
                        start=(fb == 0), stop=(fb == N_FB - 1))
                ot = otp.tile([128, SCW], BF16, tag="ot", name="ot")
                if idx % 2 == 0:
                    nc.vector.tensor_copy(ot[:], ps[:])
                    nc.sync.dma_start(
                        out_d[sb * 128:(sb + 1) * 128, mc * SCW:(mc + 1) * SCW], ot[:])
                else:
                    nc.scalar.copy(ot[:], ps[:])
                    nc.scalar.dma_start(
                        out_d[sb * 128:(sb + 1) * 128, mc * SCW:(mc + 1) * SCW], ot[:])
            return f

        def make_oproj_units(sb_list, gen):
            units = [oproj_dma(0, gen), oproj_dma(1, gen)]
            idx = 0
            for mc in range(N_MC):
                for i, sb in enumerate(sb_list):
                    units.append(oproj_mm(mc, sb, gen, idx))
                    idx += 1
                    if i == 0 and mc + 2 < N_MC:
                        units.append(oproj_dma(mc + 2, gen))
            return units

        # ---------------- filler machinery ----------------------------------
        filler = deque()

        # ---------------- attention -----------------------------------------
        def batches_for(qc):
            bs = []
            for i in range(2 * qc):
                bs.append(dict(blocks=[(2 * i, 0, 512, 0), (2 * i + 1, 512, 512, 0)],
                               width=1024, diag=[]))
            base = 4 * qc
            bs.append(dict(blocks=[(base, 0, 512, 0), (base + 1, 512, 384, 128)],
                           width=896, diag=[0, 1]))
            bs.append(dict(blocks=[(base + 2, 0, 256, 256), (base + 3, 256, 128, 384)],
                           width=384, diag=[0, 1]))
            return bs

        def emit_attn(qc, hold=0):
            bs_proto = batches_for(qc)
            # per-drain-slot weights: attention PE-ns emitted before the slot
            weights = []
            for h in range(NQ):
                for b in bs_proto:
                    weights.append(sum(bl[2] for bl in b["blocks"]) * 0.43)
                for j in range(4):
                    weights.append((4 * qc + j + 1) * 81.0)
                weights.append(600.0)   # transpose group slot
            total_w = sum(weights)
            nfill0 = len(filler)
            st_drain = dict(done=0, si=0, cum=0.0)

            def drain():
                st_drain["cum"] += weights[st_drain["si"]]
                st_drain["si"] += 1
                target = min(round(nfill0 * st_drain["cum"] / total_w),
                             max(0, nfill0 - hold))
                while st_drain["done"] < target and filler:
                    filler.popleft()()
                    st_drain["done"] += 1

            for h in range(NQ):
                pt_map = {}
                for b in bs_proto:
                    sc_t = ps_sc.tile([128, 1024], F32, tag="sc", name="sc")
                    for (kb, off, w, q_lo) in b["blocks"]:
                        nc.tensor.matmul(
                            sc_t[:, off:off + w],
                            lhsT=KT[kb // 4][:, (kb % 4) * 128:(kb % 4) * 128 + 128],
                            rhs=QT[h][qc][:, q_lo:q_lo + w],
                            start=True, stop=True)
                    wdt = b["width"]
                    st = stp.tile([128, 1024], BF16, tag="st", name="st")
                    nc.scalar.activation(st[:, :wdt], sc_t[:, :wdt], AF.Tanh,
                                         scale=SCALE / SOFTCAP)
                    pt = ptp.tile([128, 1024], BF16, tag="pt", name="pt")
                    nc.scalar.activation(pt[:, :wdt], st[:, :wdt], AF.Exp,
                                         scale=SOFTCAP, bias=negcap[:])
                    for bi in b["diag"]:
                        (kb, off, w, q_lo) = b["blocks"][bi]
                        g = kb - 4 * qc
                        dcol = off + (g * 128 - q_lo)
                        nc.vector.tensor_mul(pt[:, dcol:dcol + 128],
                                             pt[:, dcol:dcol + 128], triu[:])
                    for (kb, off, w, q_lo) in b["blocks"]:
                        pt_map[kb] = (pt, off, q_lo)
                    drain()
                ns_h = []
                for j in range(4):
                    qb = 4 * qc + j
                    pv = ps_pv.tile([128, 129], F32, tag="pv", name="pv")
                    for kb in range(qb + 1):
                        pt, off, q_lo = pt_map[kb]
                        col = off + (j * 128 - q_lo)
                        nc.tensor.matmul(
                            pv[:],
                            lhsT=pt[:, col:col + 128],
                            rhs=VN[kb // 4][:, (kb % 4) * 129:(kb % 4) * 129 + 129],
                            start=(kb == 0), stop=(kb == qb))
                    r = rp.tile([128, 1], F32, tag="r", name="r")
                    nc.vector.reciprocal(r[:], pv[:, 128:129])
                    n = nsp.tile([128, 128], BF16, tag="ns", name="ns")
                    nc.vector.tensor_scalar_mul(n[:], pv[:, 0:128], r[:])
                    ns_h.append(n)
                    drain()
                drain()
                # transpose this head's normalized output back to [d, q]
                tr = ps_sc.tile([128, 512], BF16, tag="sc", name="sc")
                for j in range(4):
                    nc.tensor.transpose(tr[:, j * 128:(j + 1) * 128],
                                        ns_h[j][:], ident[:])
                for j in range(4):
                    nc.vector.tensor_copy(AOT[h][qc][:, j * 128:(j + 1) * 128],
                                          tr[:, j * 128:(j + 1) * 128])

        # ================= emission =========================================
        # preamble: K, V and Q-head-0 projections inline; the remaining
        # chunk-0 projection units become attention-0 filler
        pre = make_qkv_units(0)
        for i, u in enumerate(pre[:12]):
            u()
            if i == 1:
                # rope tables only needed from the first rope (~20us in);
                # deferring them keeps early DMA bandwidth for ht/wq
                nc.sync.dma_start(cosf[:], cosf_d[:])
                nc.sync.dma_start(sinf[:], sinf_d[:])
        filler.extend(pre[12:])
        for qc in range(N_SC):
            if qc + 1 < N_SC:
                emit_ht_dma(qc + 1)
                filler.extend(make_qkv_units(qc + 1))
            else:
                filler.extend(make_oproj_units(list(range(12)), gen=0))
            # hold a few filler units back across the boundary so the next
            # chunk's scalar-bound first head still has tensor work queued
            emit_attn(qc, hold=0 if qc == N_SC - 1 else 6)
            if qc == N_SC - 1:
                while filler:
                    filler.popleft()()
        for u in make_oproj_units([12, 13, 14, 15], gen=1):
            u()

    nc.compile()
    return nc


def prep_inputs(positions, hidden_states, w_qkv, w_o):
    """Host-side shard + relayout. Returns per-core input maps."""
    bf = ml_dtypes.bfloat16
    pos = np.asarray(positions).astype(np.float32)
    hidden = np.ascontiguousarray(np.asarray(hidden_states, dtype=np.float32))
    w_qkv = np.asarray(w_qkv, dtype=np.float32)
    w_o = np.asarray(w_o, dtype=np.float32)

    # rope tables (neox): freqs [S, 64]
    inv_freq = 1.0 / (ROPE_THETA ** (np.arange(0, D, 2, dtype=np.float32) / D))
    freqs = pos[:, None] * inv_freq[None, :]
    cos = np.cos(freqs).T.astype(np.float32)   # [64, S]
    sin = np.sin(freqs).T.astype(np.float32)
    cosf = np.concatenate([cos, cos], axis=0)               # [128, S]
    sinf = np.concatenate([-sin, sin], axis=0)

    triu = np.triu(np.ones((128, 128), np.float32)).astype(bf)  # [k, q]: q >= k
    ident = np.eye(128, dtype=np.float32).astype(bf)

    # ht[sc, hb, p, c] = hidden[sc*512+c, hb*128+p], regrouped 4 hb per tile:
    # htg[sc, g, p, g2*512+c] = ht[sc, 4g+g2, p, c]
    ht = np.ascontiguousarray(
        hidden.reshape(N_SC, SCW, N_HB, 128).transpose(0, 2, 3, 1)
        .reshape(N_SC, 12, 4, 128, SCW).transpose(0, 1, 3, 2, 4)
        .reshape(N_SC, 12, 128, 4 * SCW)).astype(bf)

    in_maps = []
    for c in range(N_CORES):
        q_rows = w_qkv[c * NQ * D:(c + 1) * NQ * D]          # [768, 6144]
        k_rows = w_qkv[HID + c * D:HID + (c + 1) * D]        # [128, 6144]
        v_rows = w_qkv[HID + 8 * D + c * D:HID + 8 * D + (c + 1) * D]
        wq_c = np.concatenate([q_rows, k_rows, v_rows], axis=0)  # [1024, 6144]
        # wq[ob, p, hb, o] = wq_c[ob*128+o, hb*128+p]
        wq_arr = np.ascontiguousarray(
            wq_c.reshape(N_OB, 128, N_HB, 128).transpose(0, 3, 2, 1)).astype(bf)
        wo_c = (w_o[:, c * NQ * D:(c + 1) * NQ * D] * ATTN_MULT).T  # [768, 6144]
        # wo[mc, p, fb, m] = wo_c[fb*128+p, mc*512+m]
        wo_arr = np.ascontiguousarray(
            wo_c.reshape(N_FB, 128, N_MC, SCW).transpose(2, 1, 0, 3)).astype(bf)
        in_maps.append({
            "ht": ht, "wq": wq_arr, "wo": wo_arr,
            "cosf": cosf, "sinf": sinf, "triu": triu, "ident": ident,
            "negcap": np.full((128, 1), -SOFTCAP, np.float32),
        })
    return in_maps


_NC_CACHE = None


def _get_nc():
    global _NC_CACHE
    if _NC_CACHE is None:
        _NC_CACHE = build_nc()
    return _NC_CACHE


def kernel(positions, hidden_states, w_qkv, w_o, _trace=False, _trace_kwargs=None):
    nc = _get_nc()
    in_maps = prep_inputs(positions, hidden_states, w_qkv, w_o)
    res = run_bass_kernel_spmd(nc, in_maps, list(range(N_CORES)),
                               trace=_trace, **(_trace_kwargs or {}))
    out = np.zeros((S, HID), np.float32)
    for c in range(N_CORES):
        out += res.results[c]["out"].astype(np.float32)
    out = out.astype(np.asarray(hidden_states).dtype)
    kernel.last_results = res
    return out


# revision 47
# speedup vs baseline: 1.0116x; 1.0033x over previous
"""Grok1-style GQA attention (S=2048, H=6144, 48 Q heads / 8 KV heads, rope,
softcap-30, causal) as a Bass/Tile kernel sharded over 8 NeuronCores.

Sharding: tensor-parallel across heads. Core c owns Q heads 6c..6c+5 and KV
head c. Each core computes its qkv projection slice, rope, causal softcap
attention for its 6 Q heads against its single KV head, and a partial
o_proj (its 768 columns of w_o). The host sums the 8 partial outputs.

Numerics: softcap bounds scores to [-30, 30], so softmax is computed as
exp(30*tanh(s/30) - 30) with a *constant* bias — no running max.

v2 design (vs the naive per-block version):
 - The softmax denominator comes free from the PV matmul: V is augmented
   with a ones column (VN blocks are [k,129], col 128 = 1), and PV is done
   in [q, d] orientation (lhsT = probs [k,q], rhs = V_aug [k,129]) so the
   per-query denominator lands on the q PARTITION axis -> cheap per-partition
   reciprocal + scale on the vector engine. This kills the M=1 row-sum
   matmuls, the K=1 broadcast matmuls and the single-lane reciprocals.
 - tanh/exp run on [128, <=1024] batches spanning 2 PSUM banks (fewer ACT
   instructions, less fixed overhead).
 - Software pipelining by emission order: attention of chunk i is
   interleaved with the QKV projection of chunk i+1 (and attention of the
   last chunk with the first 3/4 of o_proj) so the tensor engine never
   stalls on the scalar engine and HAM stays warm.
 - Normalized attention outputs are transposed back to [d, q] in bulk at
   chunk end (PE transpose + DVE copy) for the o_proj lhsT.

Layouts (host-prepped, contraction dim on SBUF partitions):
  ht   [4,48,128,512] bf16  : ht[sc,hb,p,c] = hidden[sc*512+c, hb*128+p]
  wq   [8,128,48,128] bf16  : wq[ob,p,hb,o] = w_qkv_core[ob*128+o, hb*128+p]
  wo   [12,128,6,512] bf16  : wo[mc,p,fb,m] = (w_o[:,core]*MULT).T[fb*128+p, mc*512+m]
  cosf/sinf [128,2048] f32  : duplicated/sign-flipped rope tables (neox)
  triu [128,128] bf16       : triu[k,q] = 1 if q >= k else 0
"""

import sys
import numpy as np
from collections import deque

sys.path.insert(0, "/opt/trn_rl_repo")

import ml_dtypes

import concourse.bass as bass
import concourse.mybir as mybir
import concourse.tile as tile
from concourse import bacc
from concourse.bass_utils import run_bass_kernel_spmd

F32 = mybir.dt.float32
BF16 = mybir.dt.bfloat16
AF = mybir.ActivationFunctionType

S = 2048
HID = 6144
D = 128
NQ = 6          # q heads per core
N_CORES = 8
SCALE = D ** -0.5
SOFTCAP = 30.0
ATTN_MULT = 0.08838834764831845
ROPE_THETA = 10000.0

N_SC = 4        # s-chunks of 512
SCW = 512
N_HB = 48       # hidden 128-blocks
N_OB = 8        # output 128-blocks per core (6 Q | 1 K | 1 V)
N_MC = 12       # o_proj 512-col chunks
N_SB = 16       # s 128-blocks
N_FB = 6        # per-core o_proj feature 128-blocks (768/128)


def build_nc():
    nc = bacc.Bacc("TRN2", target_bir_lowering=False, debug=False, num_devices=N_CORES)

    ht_d = nc.dram_tensor("ht", [N_SC, 12, 128, 4 * SCW], BF16, kind="ExternalInput").ap()
    wq_d = nc.dram_tensor("wq", [N_OB, 128, N_HB, 128], BF16, kind="ExternalInput").ap()
    wo_d = nc.dram_tensor("wo", [N_MC, 128, N_FB, SCW], BF16, kind="ExternalInput").ap()
    cosf_d = nc.dram_tensor("cosf", [128, S], F32, kind="ExternalInput").ap()
    sinf_d = nc.dram_tensor("sinf", [128, S], F32, kind="ExternalInput").ap()
    triu_d = nc.dram_tensor("triu", [128, 128], BF16, kind="ExternalInput").ap()
    ident_d = nc.dram_tensor("ident", [128, 128], BF16, kind="ExternalInput").ap()
    negcap_d = nc.dram_tensor("negcap", [128, 1], F32, kind="ExternalInput").ap()
    out_d = nc.dram_tensor("out", [S, HID], BF16, kind="ExternalOutput").ap()

    from contextlib import ExitStack
    with tile.TileContext(nc) as tc, ExitStack() as ctx:
        const = ctx.enter_context(tc.tile_pool(name="const", bufs=1))
        pers = ctx.enter_context(tc.tile_pool(name="pers", bufs=1))
        htp = ctx.enter_context(tc.tile_pool(name="htp", bufs=12))
        wqp = ctx.enter_context(tc.tile_pool(name="wqp", bufs=2))
        wop = ctx.enter_context(tc.tile_pool(name="wop", bufs=2))
        ropep = ctx.enter_context(tc.tile_pool(name="ropep", bufs=4))
        stp = ctx.enter_context(tc.tile_pool(name="stp", bufs=2))
        ptp = ctx.enter_context(tc.tile_pool(name="ptp", bufs=8))
        nsp = ctx.enter_context(tc.tile_pool(name="nsp", bufs=6))
        rp = ctx.enter_context(tc.tile_pool(name="rp", bufs=4))
        otp = ctx.enter_context(tc.tile_pool(name="otp", bufs=4))
        ps_a = ctx.enter_context(tc.tile_pool(name="ps_a", bufs=2, space=bass.MemorySpace.PSUM))
        ps_sc = ctx.enter_context(tc.tile_pool(name="ps_sc", bufs=2, space=bass.MemorySpace.PSUM))
        ps_pv = ctx.enter_context(tc.tile_pool(name="ps_pv", bufs=2, space=bass.MemorySpace.PSUM))

        # ---------- persistent SBUF tiles (per s-chunk for precise deps) ----
        QT = [[pers.tile([128, SCW], BF16, tag=f"qt{h}_{c}", name=f"qt{h}_{c}")
               for c in range(N_SC)] for h in range(NQ)]
        KT = [pers.tile([128, SCW], BF16, tag=f"kt{c}", name=f"kt{c}") for c in range(N_SC)]
        VT = [pers.tile([128, SCW], BF16, tag=f"vt{c}", name=f"vt{c}") for c in range(N_SC)]
        VN = [pers.tile([128, 4 * 129], BF16, tag=f"vn{c}", name=f"vn{c}") for c in range(N_SC)]
        AOT = [[pers.tile([128, SCW], BF16, tag=f"aot{h}_{c}", name=f"aot{h}_{c}")
                for c in range(N_SC)] for h in range(NQ)]

        ht_tiles = {}

        def emit_ht_dma(sc, fine=False):
            lst = []
            for g in range(12):
                t = htp.tile([128, 4 * SCW], BF16, tag="ht", name="ht")
                if fine:
                    # chunk 0: split per-hb so the first matmul starts sooner
                    for g2 in range(4):
                        nc.sync.dma_start(t[:, g2 * SCW:(g2 + 1) * SCW],
                                          ht_d[sc, g][:, g2 * SCW:(g2 + 1) * SCW])
                else:
                    nc.sync.dma_start(t[:], ht_d[sc, g])
                lst.append(t)
            ht_tiles[sc] = lst

        # hidden chunk 0 first so the first matmul can start ASAP
        emit_ht_dma(0)

        cosf = const.tile([128, S], F32, tag="cosf", name="cosf")
        sinf = const.tile([128, S], F32, tag="sinf", name="sinf")
        triu = const.tile([128, 128], BF16, tag="triu", name="triu")
        ident = const.tile([128, 128], BF16, tag="ident", name="ident")
        negcap = const.tile([128, 1], F32, tag="negcap", name="negcap")
        nc.sync.dma_start(triu[:], triu_d[:])
        nc.sync.dma_start(ident[:], ident_d[:])
        nc.sync.dma_start(negcap[:], negcap_d[:])

        # ---------------- QKV projection units (4 per ob) -------------------
        def make_qkv_units(sc):
            state = {}
            wtiles = {}
            done = set()
            scs = slice(sc * SCW, (sc + 1) * SCW)

            def issue_wq(ob, qds):
                if ob not in wtiles:
                    wtiles[ob] = (wqp.tile([128, N_HB * 128], BF16,
                                           tag="wq", name="wq"), set())
                w, issued = wtiles[ob]
                for qd in qds:
                    if qd not in issued:
                        issued.add(qd)
                        nc.gpsimd.dma_start(
                            w[:, qd * 1536:(qd + 1) * 1536],
                            wq_d[ob, :, qd * 12:(qd + 1) * 12])
                return w

            def ensure_dma():
                # pre-issue any outstanding weight transfers so held-back
                # units don't stall on DMA at the next chunk boundary
                for ob in (6, 7, 0, 1, 2, 3, 4, 5):
                    if ob not in done:
                        issue_wq(ob, (0, 1, 2, 3))

            def unit(ob, part):
                if part == 0:
                    w = issue_wq(ob, (0, 1))
                    state[ob] = ps_a.tile([128, SCW], F32, tag="acc", name="acc")
                ps = state[ob]
                if part == 2:
                    w = issue_wq(ob, (2, 3))
                w = wtiles[ob][0]
                for hb in range(part * 12, part * 12 + 12):
                    nc.tensor.matmul(
                        ps[:], lhsT=w[:, hb * 128:(hb + 1) * 128],
                        rhs=ht_tiles[sc][hb // 4][:, (hb % 4) * SCW:(hb % 4 + 1) * SCW],
                        start=(hb == 0), stop=(hb == N_HB - 1))
                if part != 3:
                    return
                done.add(ob)
                state.pop(ob)
                wtiles.pop(ob)
                if ob == 7:
                    nc.vector.tensor_copy(VT[sc][:], ps[:])
                    nc.vector.memset(VN[sc][:], 1.0)
                    tr = ps_sc.tile([128, 1024], BF16, tag="sc", name="sc")
                    for j in range(4):
                        nc.tensor.transpose(
                            tr[:, j * 128:(j + 1) * 128],
                            VT[sc][:, j * 128:(j + 1) * 128], ident[:])
                    for j in range(4):
                        nc.vector.tensor_copy(
                            VN[sc][:, j * 129:j * 129 + 128],
                            tr[:, j * 128:(j + 1) * 128])
                else:
                    rot = ropep.tile([128, SCW], F32, tag="rot", name="rot")
                    t1 = ropep.tile([128, SCW], F32, tag="t1", name="t1")
                    nc.scalar.copy(rot[0:64, :], ps[64:128, :])
                    nc.scalar.copy(rot[64:128, :], ps[0:64, :])
                    nc.vector.tensor_mul(t1[:], ps[:], cosf[:, scs])
                    nc.vector.tensor_mul(rot[:], rot[:], sinf[:, scs])
                    dst = QT[ob][sc] if ob < NQ else KT[sc]
                    nc.vector.tensor_add(dst[:], t1[:], rot[:])

            units = []
            for ob in (6, 7, 0, 1, 2, 3, 4, 5):   # K, V first, then Q heads
                for part in range(4):
                    units.append(lambda ob=ob, part=part: unit(ob, part))
            return units, ensure_dma

        # ---------------- o_proj units --------------------------------------
        wo_state = {}

        def oproj_dma(mc, gen):
            def f():
                w = wop.tile([128, N_FB * SCW], BF16, tag="wo", name="wo")
                nc.gpsimd.dma_start(w[:], wo_d[mc])
                wo_state[(mc, gen)] = w
            return f

        def oproj_mm(mc, sb, gen, idx):
            def f():
                w = wo_state[(mc, gen)]
                if gen == 1 and idx % 3 == 0:
                    ps = ps_sc.tile([128, SCW], F32, tag="sc", name="sc")
                elif gen == 1 and idx % 3 == 1:
                    ps = ps_pv.tile([128, SCW], F32, tag="pv", name="pv")
                else:
                    ps = ps_a.tile([128, SCW], F32, tag="acc", name="acc")
                for fb in range(N_FB):
                    nc.tensor.matmul(
                        ps[:],
                        lhsT=AOT[fb][sb // 4][:, (sb % 4) * 128:(sb % 4) * 128 + 128],
                        rhs=w[:, fb * SCW:(fb + 1) * SCW],
                        start=(fb == 0), stop=(fb == N_FB - 1))
                ot = otp.tile([128, SCW], BF16, tag="ot", name="ot")
                if idx % 2 == 0:
                    nc.vector.tensor_copy(ot[:], ps[:])
                    nc.sync.dma_start(
                        out_d[sb * 128:(sb + 1) * 128, mc * SCW:(mc + 1) * SCW], ot[:])
                else:
                    nc.scalar.copy(ot[:], ps[:])
                    nc.scalar.dma_start(
                        out_d[sb * 128:(sb + 1) * 128, mc * SCW:(mc + 1) * SCW], ot[:])
            return f

        def make_oproj_units(sb_list, gen):
            units = [oproj_dma(0, gen), oproj_dma(1, gen)]
            idx = 0
            for mc in range(N_MC):
                for i, sb in enumerate(sb_list):
                    units.append(oproj_mm(mc, sb, gen, idx))
                    idx += 1
                    if i == 0 and mc + 2 < N_MC:
                        units.append(oproj_dma(mc + 2, gen))
            return units

        # ---------------- filler machinery ----------------------------------
        filler = deque()

        # ---------------- attention -----------------------------------------
        def batches_for(qc):
            bs = []
            for i in range(2 * qc):
                bs.append(dict(blocks=[(2 * i, 0, 512, 0), (2 * i + 1, 512, 512, 0)],
                               width=1024, diag=[]))
            base = 4 * qc
            bs.append(dict(blocks=[(base, 0, 512, 0), (base + 1, 512, 384, 128)],
                           width=896, diag=[0, 1]))
            bs.append(dict(blocks=[(base + 2, 0, 256, 256), (base + 3, 256, 128, 384)],
                           width=384, diag=[0, 1]))
            return bs

        def emit_attn(qc, hold=0):
            bs_proto = batches_for(qc)
            # per-drain-slot weights: attention PE-ns emitted before the slot
            weights = []
            for h in range(NQ):
                for b in bs_proto:
                    weights.append(sum(bl[2] for bl in b["blocks"]) * 0.43)
                for j in range(4):
                    weights.append((4 * qc + j + 1) * 81.0)
                weights.append(600.0)   # transpose group slot
            total_w = sum(weights)
            nfill0 = len(filler)
            st_drain = dict(done=0, si=0, cum=0.0)

            def drain():
                st_drain["cum"] += weights[st_drain["si"]]
                st_drain["si"] += 1
                target = min(round(nfill0 * st_drain["cum"] / total_w),
                             max(0, nfill0 - hold))
                while st_drain["done"] < target and filler:
                    filler.popleft()()
                    st_drain["done"] += 1

            for h in range(NQ):
                pt_map = {}
                for b in bs_proto:
                    sc_t = ps_sc.tile([128, 1024], F32, tag="sc", name="sc")
                    for (kb, off, w, q_lo) in b["blocks"]:
                        nc.tensor.matmul(
                            sc_t[:, off:off + w],
                            lhsT=KT[kb // 4][:, (kb % 4) * 128:(kb % 4) * 128 + 128],
                            rhs=QT[h][qc][:, q_lo:q_lo + w],
                            start=True, stop=True)
                    wdt = b["width"]
                    st = stp.tile([128, 1024], BF16, tag="st", name="st")
                    nc.scalar.activation(st[:, :wdt], sc_t[:, :wdt], AF.Tanh,
                                         scale=SCALE / SOFTCAP)
                    pt = ptp.tile([128, 1024], BF16, tag="pt", name="pt")
                    nc.scalar.activation(pt[:, :wdt], st[:, :wdt], AF.Exp,
                                         scale=SOFTCAP, bias=negcap[:])
                    for bi in b["diag"]:
                        (kb, off, w, q_lo) = b["blocks"][bi]
                        g = kb - 4 * qc
                        dcol = off + (g * 128 - q_lo)
                        nc.vector.tensor_mul(pt[:, dcol:dcol + 128],
                                             pt[:, dcol:dcol + 128], triu[:])
                    for (kb, off, w, q_lo) in b["blocks"]:
                        pt_map[kb] = (pt, off, q_lo)
                    drain()
                ns_h = []
                for j in range(4):
                    qb = 4 * qc + j
                    pv = ps_pv.tile([128, 129], F32, tag="pv", name="pv")
                    for kb in range(qb + 1):
                        pt, off, q_lo = pt_map[kb]
                        col = off + (j * 128 - q_lo)
                        nc.tensor.matmul(
                            pv[:],
                            lhsT=pt[:, col:col + 128],
                            rhs=VN[kb // 4][:, (kb % 4) * 129:(kb % 4) * 129 + 129],
                            start=(kb == 0), stop=(kb == qb))
                    r = rp.tile([128, 1], F32, tag="r", name="r")
                    nc.vector.reciprocal(r[:], pv[:, 128:129])
                    n = nsp.tile([128, 128], BF16, tag="ns", name="ns")
                    nc.vector.tensor_scalar_mul(n[:], pv[:, 0:128], r[:])
                    ns_h.append(n)
                    drain()
                drain()
                # transpose this head's normalized output back to [d, q]
                tr = ps_sc.tile([128, 512], BF16, tag="sc", name="sc")
                for j in range(4):
                    nc.tensor.transpose(tr[:, j * 128:(j + 1) * 128],
                                        ns_h[j][:], ident[:])
                for j in range(4):
                    nc.vector.tensor_copy(AOT[h][qc][:, j * 128:(j + 1) * 128],
                                          tr[:, j * 128:(j + 1) * 128])

        # ================= emission =========================================
        # preamble: K, V and Q-head-0 projections inline; the remaining
        # chunk-0 projection units become attention-0 filler
        pre, _ = make_qkv_units(0)
        for i, u in enumerate(pre[:12]):
            u()
            if i == 1:
                # rope tables only needed from the first rope (~20us in);
                # deferring them keeps early DMA bandwidth for ht/wq
                nc.sync.dma_start(cosf[:], cosf_d[:])
                nc.sync.dma_start(sinf[:], sinf_d[:])
        filler.extend(pre[12:])
        for qc in range(N_SC):
            ensure = None
            if qc + 1 < N_SC:
                emit_ht_dma(qc + 1)
                units, ensure = make_qkv_units(qc + 1)
                filler.extend(units)
            else:
                filler.extend(make_oproj_units(list(range(12)), gen=0))
            # hold a few filler units back across the boundary so the next
            # chunk's scalar-bound first head still has tensor work queued
            emit_attn(qc, hold=0 if qc == N_SC - 1 else 8)
            if ensure is not None:
                ensure()
            if qc == N_SC - 1:
                while filler:
                    filler.popleft()()
        for u in make_oproj_units([12, 13, 14, 15], gen=1):
            u()

    nc.compile()
    return nc


def prep_inputs(positions, hidden_states, w_qkv, w_o):
    """Host-side shard + relayout. Returns per-core input maps."""
    bf = ml_dtypes.bfloat16
    pos = np.asarray(positions).astype(np.float32)
    hidden = np.ascontiguousarray(np.asarray(hidden_states, dtype=np.float32))
    w_qkv = np.asarray(w_qkv, dtype=np.float32)
    w_o = np.asarray(w_o, dtype=np.float32)

    # rope tables (neox): freqs [S, 64]
    inv_freq = 1.0 / (ROPE_THETA ** (np.arange(0, D, 2, dtype=np.float32) / D))
    freqs = pos[:, None] * inv_freq[None, :]
    cos = np.cos(freqs).T.astype(np.float32)   # [64, S]
    sin = np.sin(freqs).T.astype(np.float32)
    cosf = np.concatenate([cos, cos], axis=0)               # [128, S]
    sinf = np.concatenate([-sin, sin], axis=0)

    triu = np.triu(np.ones((128, 128), np.float32)).astype(bf)  # [k, q]: q >= k
    ident = np.eye(128, dtype=np.float32).astype(bf)

    # ht[sc, hb, p, c] = hidden[sc*512+c, hb*128+p], regrouped 4 hb per tile:
    # htg[sc, g, p, g2*512+c] = ht[sc, 4g+g2, p, c]
    ht = np.ascontiguousarray(
        hidden.reshape(N_SC, SCW, N_HB, 128).transpose(0, 2, 3, 1)
        .reshape(N_SC, 12, 4, 128, SCW).transpose(0, 1, 3, 2, 4)
        .reshape(N_SC, 12, 128, 4 * SCW)).astype(bf)

    in_maps = []
    for c in range(N_CORES):
        q_rows = w_qkv[c * NQ * D:(c + 1) * NQ * D]          # [768, 6144]
        k_rows = w_qkv[HID + c * D:HID + (c + 1) * D]        # [128, 6144]
        v_rows = w_qkv[HID + 8 * D + c * D:HID + 8 * D + (c + 1) * D]
        wq_c = np.concatenate([q_rows, k_rows, v_rows], axis=0)  # [1024, 6144]
        # wq[ob, p, hb, o] = wq_c[ob*128+o, hb*128+p]
        wq_arr = np.ascontiguousarray(
            wq_c.reshape(N_OB, 128, N_HB, 128).transpose(0, 3, 2, 1)).astype(bf)
        wo_c = (w_o[:, c * NQ * D:(c + 1) * NQ * D] * ATTN_MULT).T  # [768, 6144]
        # wo[mc, p, fb, m] = wo_c[fb*128+p, mc*512+m]
        wo_arr = np.ascontiguousarray(
            wo_c.reshape(N_FB, 128, N_MC, SCW).transpose(2, 1, 0, 3)).astype(bf)
        in_maps.append({
            "ht": ht, "wq": wq_arr, "wo": wo_arr,
            "cosf": cosf, "sinf": sinf, "triu": triu, "ident": ident,
            "negcap": np.full((128, 1), -SOFTCAP, np.float32),
        })
    return in_maps


_NC_CACHE = None


def _get_nc():
    global _NC_CACHE
    if _NC_CACHE is None:
        _NC_CACHE = build_nc()
    return _NC_CACHE


def kernel(positions, hidden_states, w_qkv, w_o, _trace=False, _trace_kwargs=None):
    nc = _get_nc()
    in_maps = prep_inputs(positions, hidden_states, w_qkv, w_o)
    res = run_bass_kernel_spmd(nc, in_maps, list(range(N_CORES)),
                               trace=_trace, **(_trace_kwargs or {}))
    out = np.zeros((S, HID), np.float32)
    for c in range(N_CORES):
        out += res.results[c]["out"].astype(np.float32)
    out = out.astype(np.asarray(hidden_states).dtype)
    kernel.last_results = res
    return out


# revision 50
# speedup vs baseline: 1.0146x; 1.0030x over previous
"""Grok1-style GQA attention (S=2048, H=6144, 48 Q heads / 8 KV heads, rope,
softcap-30, causal) as a Bass/Tile kernel sharded over 8 NeuronCores.

Sharding: tensor-parallel across heads. Core c owns Q heads 6c..6c+5 and KV
head c. Each core computes its qkv projection slice, rope, causal softcap
attention for its 6 Q heads against its single KV head, and a partial
o_proj (its 768 columns of w_o). The host sums the 8 partial outputs.

Numerics: softcap bounds scores to [-30, 30], so softmax is computed as
exp(30*tanh(s/30) - 30) with a *constant* bias — no running max.

v2 design (vs the naive per-block version):
 - The softmax denominator comes free from the PV matmul: V is augmented
   with a ones column (VN blocks are [k,129], col 128 = 1), and PV is done
   in [q, d] orientation (lhsT = probs [k,q], rhs = V_aug [k,129]) so the
   per-query denominator lands on the q PARTITION axis -> cheap per-partition
   reciprocal + scale on the vector engine. This kills the M=1 row-sum
   matmuls, the K=1 broadcast matmuls and the single-lane reciprocals.
 - tanh/exp run on [128, <=1024] batches spanning 2 PSUM banks (fewer ACT
   instructions, less fixed overhead).
 - Software pipelining by emission order: attention of chunk i is
   interleaved with the QKV projection of chunk i+1 (and attention of the
   last chunk with the first 3/4 of o_proj) so the tensor engine never
   stalls on the scalar engine and HAM stays warm.
 - Normalized attention outputs are transposed back to [d, q] in bulk at
   chunk end (PE transpose + DVE copy) for the o_proj lhsT.

Layouts (host-prepped, contraction dim on SBUF partitions):
  ht   [4,48,128,512] bf16  : ht[sc,hb,p,c] = hidden[sc*512+c, hb*128+p]
  wq   [8,128,48,128] bf16  : wq[ob,p,hb,o] = w_qkv_core[ob*128+o, hb*128+p]
  wo   [12,128,6,512] bf16  : wo[mc,p,fb,m] = (w_o[:,core]*MULT).T[fb*128+p, mc*512+m]
  cosf/sinf [128,2048] f32  : duplicated/sign-flipped rope tables (neox)
  triu [128,128] bf16       : triu[k,q] = 1 if q >= k else 0
"""

import sys
import numpy as np
from collections import deque

sys.path.insert(0, "/opt/trn_rl_repo")

import ml_dtypes

import concourse.bass as bass
import concourse.mybir as mybir
import concourse.tile as tile
from concourse import bacc
from concourse.bass_utils import run_bass_kernel_spmd

F32 = mybir.dt.float32
BF16 = mybir.dt.bfloat16
AF = mybir.ActivationFunctionType

S = 2048
HID = 6144
D = 128
NQ = 6          # q heads per core
N_CORES = 8
SCALE = D ** -0.5
SOFTCAP = 30.0
ATTN_MULT = 0.08838834764831845
ROPE_THETA = 10000.0

N_SC = 4        # s-chunks of 512
SCW = 512
N_HB = 48       # hidden 128-blocks
N_OB = 8        # output 128-blocks per core (6 Q | 1 K | 1 V)
N_MC = 12       # o_proj 512-col chunks
N_SB = 16       # s 128-blocks
N_FB = 6        # per-core o_proj feature 128-blocks (768/128)


def build_nc():
    nc = bacc.Bacc("TRN2", target_bir_lowering=False, debug=False, num_devices=N_CORES)

    ht_d = nc.dram_tensor("ht", [N_SC, 12, 128, 4 * SCW], BF16, kind="ExternalInput").ap()
    wq_d = nc.dram_tensor("wq", [N_OB, 128, N_HB, 128], BF16, kind="ExternalInput").ap()
    wo_d = nc.dram_tensor("wo", [N_MC, 128, N_FB, SCW], BF16, kind="ExternalInput").ap()
    cosf_d = nc.dram_tensor("cosf", [128, S], F32, kind="ExternalInput").ap()
    sinf_d = nc.dram_tensor("sinf", [128, S], F32, kind="ExternalInput").ap()
    triu_d = nc.dram_tensor("triu", [128, 128], BF16, kind="ExternalInput").ap()
    ident_d = nc.dram_tensor("ident", [128, 128], BF16, kind="ExternalInput").ap()
    negcap_d = nc.dram_tensor("negcap", [128, 1], F32, kind="ExternalInput").ap()
    out_d = nc.dram_tensor("out", [S, HID], BF16, kind="ExternalOutput").ap()

    from contextlib import ExitStack
    with tile.TileContext(nc) as tc, ExitStack() as ctx:
        const = ctx.enter_context(tc.tile_pool(name="const", bufs=1))
        pers = ctx.enter_context(tc.tile_pool(name="pers", bufs=1))
        htp = ctx.enter_context(tc.tile_pool(name="htp", bufs=12))
        wqp = ctx.enter_context(tc.tile_pool(name="wqp", bufs=2))
        wop = ctx.enter_context(tc.tile_pool(name="wop", bufs=2))
        ropep = ctx.enter_context(tc.tile_pool(name="ropep", bufs=4))
        stp = ctx.enter_context(tc.tile_pool(name="stp", bufs=2))
        ptp = ctx.enter_context(tc.tile_pool(name="ptp", bufs=8))
        nsp = ctx.enter_context(tc.tile_pool(name="nsp", bufs=6))
        rp = ctx.enter_context(tc.tile_pool(name="rp", bufs=4))
        otp = ctx.enter_context(tc.tile_pool(name="otp", bufs=4))
        ps_a = ctx.enter_context(tc.tile_pool(name="ps_a", bufs=2, space=bass.MemorySpace.PSUM))
        ps_sc = ctx.enter_context(tc.tile_pool(name="ps_sc", bufs=2, space=bass.MemorySpace.PSUM))
        ps_pv = ctx.enter_context(tc.tile_pool(name="ps_pv", bufs=2, space=bass.MemorySpace.PSUM))

        # ---------- persistent SBUF tiles (per s-chunk for precise deps) ----
        QT = [[pers.tile([128, SCW], BF16, tag=f"qt{h}_{c}", name=f"qt{h}_{c}")
               for c in range(N_SC)] for h in range(NQ)]
        KT = [pers.tile([128, SCW], BF16, tag=f"kt{c}", name=f"kt{c}") for c in range(N_SC)]
        VT = [pers.tile([128, SCW], BF16, tag=f"vt{c}", name=f"vt{c}") for c in range(N_SC)]
        VN = [pers.tile([128, 4 * 129], BF16, tag=f"vn{c}", name=f"vn{c}") for c in range(N_SC)]
        AOT = [[pers.tile([128, SCW], BF16, tag=f"aot{h}_{c}", name=f"aot{h}_{c}")
                for c in range(N_SC)] for h in range(NQ)]

        ht_tiles = {}

        def emit_ht_dma(sc, fine=False):
            lst = []
            for g in range(12):
                t = htp.tile([128, 4 * SCW], BF16, tag="ht", name="ht")
                if fine:
                    # chunk 0: split per-hb so the first matmul starts sooner
                    for g2 in range(4):
                        nc.sync.dma_start(t[:, g2 * SCW:(g2 + 1) * SCW],
                                          ht_d[sc, g][:, g2 * SCW:(g2 + 1) * SCW])
                else:
                    nc.sync.dma_start(t[:], ht_d[sc, g])
                lst.append(t)
            ht_tiles[sc] = lst

        # hidden chunk 0 first so the first matmul can start ASAP
        emit_ht_dma(0)

        cosf = const.tile([128, S], F32, tag="cosf", name="cosf")
        sinf = const.tile([128, S], F32, tag="sinf", name="sinf")
        triu = const.tile([128, 128], BF16, tag="triu", name="triu")
        ident = const.tile([128, 128], BF16, tag="ident", name="ident")
        negcap = const.tile([128, 1], F32, tag="negcap", name="negcap")
        nc.sync.dma_start(triu[:], triu_d[:])
        nc.sync.dma_start(ident[:], ident_d[:])
        nc.sync.dma_start(negcap[:], negcap_d[:])

        # ---------------- QKV projection units (4 per ob) -------------------
        def make_qkv_units(sc):
            state = {}
            wtiles = {}
            done = set()
            scs = slice(sc * SCW, (sc + 1) * SCW)

            def issue_wq(ob, qds):
                if ob not in wtiles:
                    wtiles[ob] = (wqp.tile([128, N_HB * 128], BF16,
                                           tag="wq", name="wq"), set())
                w, issued = wtiles[ob]
                for qd in qds:
                    if qd not in issued:
                        issued.add(qd)
                        nc.gpsimd.dma_start(
                            w[:, qd * 1536:(qd + 1) * 1536],
                            wq_d[ob, :, qd * 12:(qd + 1) * 12])
                return w

            def ensure_dma():
                # pre-issue any outstanding weight transfers so held-back
                # units don't stall on DMA at the next chunk boundary
                for ob in (6, 7, 0, 1, 2, 3, 4, 5):
                    if ob not in done:
                        issue_wq(ob, (0, 1, 2, 3))

            def unit(ob, part):
                if part == 0:
                    w = issue_wq(ob, (0, 1))
                    state[ob] = ps_a.tile([128, SCW], F32, tag="acc", name="acc")
                ps = state[ob]
                if part == 2:
                    w = issue_wq(ob, (2, 3))
                w = wtiles[ob][0]
                for hb in range(part * 12, part * 12 + 12):
                    nc.tensor.matmul(
                        ps[:], lhsT=w[:, hb * 128:(hb + 1) * 128],
                        rhs=ht_tiles[sc][hb // 4][:, (hb % 4) * SCW:(hb % 4 + 1) * SCW],
                        start=(hb == 0), stop=(hb == N_HB - 1))
                if part != 3:
                    return
                done.add(ob)
                state.pop(ob)
                wtiles.pop(ob)
                if ob == 7:
                    nc.vector.tensor_copy(VT[sc][:], ps[:])
                    nc.vector.memset(VN[sc][:], 1.0)
                    tr = ps_sc.tile([128, 1024], BF16, tag="sc", name="sc")
                    for j in range(4):
                        nc.tensor.transpose(
                            tr[:, j * 128:(j + 1) * 128],
                            VT[sc][:, j * 128:(j + 1) * 128], ident[:])
                    for j in range(4):
                        nc.vector.tensor_copy(
                            VN[sc][:, j * 129:j * 129 + 128],
                            tr[:, j * 128:(j + 1) * 128])
                else:
                    rot = ropep.tile([128, SCW], F32, tag="rot", name="rot")
                    t1 = ropep.tile([128, SCW], F32, tag="t1", name="t1")
                    nc.scalar.copy(rot[0:64, :], ps[64:128, :])
                    nc.scalar.copy(rot[64:128, :], ps[0:64, :])
                    nc.vector.tensor_mul(t1[:], ps[:], cosf[:, scs])
                    nc.vector.tensor_mul(rot[:], rot[:], sinf[:, scs])
                    dst = QT[ob][sc] if ob < NQ else KT[sc]
                    nc.vector.tensor_add(dst[:], t1[:], rot[:])

            units = []
            for ob in (6, 7, 0, 1, 2, 3, 4, 5):   # K, V first, then Q heads
                for part in range(4):
                    units.append(lambda ob=ob, part=part: unit(ob, part))
            return units, ensure_dma

        # ---------------- o_proj units --------------------------------------
        wo_state = {}

        def oproj_dma(mc, gen):
            def f():
                w = wop.tile([128, N_FB * SCW], BF16, tag="wo", name="wo")
                nc.gpsimd.dma_start(w[:], wo_d[mc])
                wo_state[(mc, gen)] = w
            return f

        def oproj_mm(mc, sb, gen, idx):
            def f():
                w = wo_state[(mc, gen)]
                if gen == 1 and idx % 3 == 0:
                    ps = ps_sc.tile([128, SCW], F32, tag="sc", name="sc")
                elif gen == 1 and idx % 3 == 1:
                    ps = ps_pv.tile([128, SCW], F32, tag="pv", name="pv")
                else:
                    ps = ps_a.tile([128, SCW], F32, tag="acc", name="acc")
                for fb in range(N_FB):
                    nc.tensor.matmul(
                        ps[:],
                        lhsT=AOT[fb][sb // 4][:, (sb % 4) * 128:(sb % 4) * 128 + 128],
                        rhs=w[:, fb * SCW:(fb + 1) * SCW],
                        start=(fb == 0), stop=(fb == N_FB - 1))
                ot = otp.tile([128, SCW], BF16, tag="ot", name="ot")
                if idx % 2 == 0:
                    nc.vector.tensor_copy(ot[:], ps[:])
                    nc.sync.dma_start(
                        out_d[sb * 128:(sb + 1) * 128, mc * SCW:(mc + 1) * SCW], ot[:])
                else:
                    nc.scalar.copy(ot[:], ps[:])
                    nc.scalar.dma_start(
                        out_d[sb * 128:(sb + 1) * 128, mc * SCW:(mc + 1) * SCW], ot[:])
            return f

        def make_oproj_units(sb_list, gen):
            units = [oproj_dma(0, gen), oproj_dma(1, gen)]
            idx = 0
            for mc in range(N_MC):
                for i, sb in enumerate(sb_list):
                    units.append(oproj_mm(mc, sb, gen, idx))
                    idx += 1
                    if i == 0 and mc + 2 < N_MC:
                        units.append(oproj_dma(mc + 2, gen))
            return units

        # ---------------- filler machinery ----------------------------------
        filler = deque()

        # ---------------- attention -----------------------------------------
        def batches_for(qc):
            bs = []
            for i in range(2 * qc):
                bs.append(dict(blocks=[(2 * i, 0, 512, 0), (2 * i + 1, 512, 512, 0)],
                               width=1024, diag=[]))
            base = 4 * qc
            bs.append(dict(blocks=[(base, 0, 512, 0), (base + 1, 512, 384, 128)],
                           width=896, diag=[0, 1]))
            bs.append(dict(blocks=[(base + 2, 0, 256, 256), (base + 3, 256, 128, 384)],
                           width=384, diag=[0, 1]))
            return bs

        def emit_attn(qc, hold=0, ensure=None):
            bs_proto = batches_for(qc)
            # per-drain-slot weights: attention PE-ns emitted before the slot
            weights = []
            for h in range(NQ):
                for b in bs_proto:
                    weights.append(sum(bl[2] for bl in b["blocks"]) * 0.43)
                for j in range(4):
                    weights.append((4 * qc + j + 1) * 81.0)
                weights.append(600.0)   # transpose group slot
            total_w = sum(weights)
            nfill0 = len(filler)
            st_drain = dict(done=0, si=0, cum=0.0)

            def drain():
                st_drain["cum"] += weights[st_drain["si"]]
                st_drain["si"] += 1
                target = min(round(nfill0 * st_drain["cum"] / total_w),
                             max(0, nfill0 - hold))
                while st_drain["done"] < target and filler:
                    filler.popleft()()
                    st_drain["done"] += 1

            for h in range(NQ):
                if h == NQ - 1 and ensure is not None:
                    # issue any outstanding next-chunk weight DMAs now so the
                    # transfers complete before the held-back units drain
                    ensure()
                pt_map = {}
                for b in bs_proto:
                    sc_t = ps_sc.tile([128, 1024], F32, tag="sc", name="sc")
                    for (kb, off, w, q_lo) in b["blocks"]:
                        nc.tensor.matmul(
                            sc_t[:, off:off + w],
                            lhsT=KT[kb // 4][:, (kb % 4) * 128:(kb % 4) * 128 + 128],
                            rhs=QT[h][qc][:, q_lo:q_lo + w],
                            start=True, stop=True)
                    wdt = b["width"]
                    st = stp.tile([128, 1024], BF16, tag="st", name="st")
                    nc.scalar.activation(st[:, :wdt], sc_t[:, :wdt], AF.Tanh,
                                         scale=SCALE / SOFTCAP)
                    pt = ptp.tile([128, 1024], BF16, tag="pt", name="pt")
                    nc.scalar.activation(pt[:, :wdt], st[:, :wdt], AF.Exp,
                                         scale=SOFTCAP, bias=negcap[:])
                    for bi in b["diag"]:
                        (kb, off, w, q_lo) = b["blocks"][bi]
                        g = kb - 4 * qc
                        dcol = off + (g * 128 - q_lo)
                        nc.vector.tensor_mul(pt[:, dcol:dcol + 128],
                                             pt[:, dcol:dcol + 128], triu[:])
                    for (kb, off, w, q_lo) in b["blocks"]:
                        pt_map[kb] = (pt, off, q_lo)
                    drain()
                ns_h = []
                for j in range(4):
                    qb = 4 * qc + j
                    pv = ps_pv.tile([128, 129], F32, tag="pv", name="pv")
                    for kb in range(qb + 1):
                        pt, off, q_lo = pt_map[kb]
                        col = off + (j * 128 - q_lo)
                        nc.tensor.matmul(
                            pv[:],
                            lhsT=pt[:, col:col + 128],
                            rhs=VN[kb // 4][:, (kb % 4) * 129:(kb % 4) * 129 + 129],
                            start=(kb == 0), stop=(kb == qb))
                    r = rp.tile([128, 1], F32, tag="r", name="r")
                    nc.vector.reciprocal(r[:], pv[:, 128:129])
                    n = nsp.tile([128, 128], BF16, tag="ns", name="ns")
                    nc.vector.tensor_scalar_mul(n[:], pv[:, 0:128], r[:])
                    ns_h.append(n)
                    drain()
                drain()
                # transpose this head's normalized output back to [d, q]
                tr = ps_sc.tile([128, 512], BF16, tag="sc", name="sc")
                for j in range(4):
                    nc.tensor.transpose(tr[:, j * 128:(j + 1) * 128],
                                        ns_h[j][:], ident[:])
                for j in range(4):
                    nc.vector.tensor_copy(AOT[h][qc][:, j * 128:(j + 1) * 128],
                                          tr[:, j * 128:(j + 1) * 128])

        # ================= emission =========================================
        # preamble: K, V and Q-head-0 projections inline; the remaining
        # chunk-0 projection units become attention-0 filler
        pre, _ = make_qkv_units(0)
        for i, u in enumerate(pre[:12]):
            u()
            if i == 1:
                # rope tables only needed from the first rope (~20us in);
                # deferring them keeps early DMA bandwidth for ht/wq
                nc.sync.dma_start(cosf[:], cosf_d[:])
                nc.sync.dma_start(sinf[:], sinf_d[:])
        filler.extend(pre[12:])
        for qc in range(N_SC):
            ensure = None
            if qc + 1 < N_SC:
                emit_ht_dma(qc + 1)
                units, ensure = make_qkv_units(qc + 1)
                filler.extend(units)
            else:
                filler.extend(make_oproj_units(list(range(12)), gen=0))
            # hold a few filler units back across the boundary so the next
            # chunk's scalar-bound first head still has tensor work queued
            emit_attn(qc, hold=0 if qc == N_SC - 1 else 8, ensure=ensure)
            if qc == N_SC - 1:
                while filler:
                    filler.popleft()()
        for u in make_oproj_units([12, 13, 14, 15], gen=1):
            u()

    nc.compile()
    return nc


def prep_inputs(positions, hidden_states, w_qkv, w_o):
    """Host-side shard + relayout. Returns per-core input maps."""
    bf = ml_dtypes.bfloat16
    pos = np.asarray(positions).astype(np.float32)
    hidden = np.ascontiguousarray(np.asarray(hidden_states, dtype=np.float32))
    w_qkv = np.asarray(w_qkv, dtype=np.float32)
    w_o = np.asarray(w_o, dtype=np.float32)

    # rope tables (neox): freqs [S, 64]
    inv_freq = 1.0 / (ROPE_THETA ** (np.arange(0, D, 2, dtype=np.float32) / D))
    freqs = pos[:, None] * inv_freq[None, :]
    cos = np.cos(freqs).T.astype(np.float32)   # [64, S]
    sin = np.sin(freqs).T.astype(np.float32)
    cosf = np.concatenate([cos, cos], axis=0)               # [128, S]
    sinf = np.concatenate([-sin, sin], axis=0)

    triu = np.triu(np.ones((128, 128), np.float32)).astype(bf)  # [k, q]: q >= k
    ident = np.eye(128, dtype=np.float32).astype(bf)

    # ht[sc, hb, p, c] = hidden[sc*512+c, hb*128+p], regrouped 4 hb per tile:
    # htg[sc, g, p, g2*512+c] = ht[sc, 4g+g2, p, c]
    ht = np.ascontiguousarray(
        hidden.reshape(N_SC, SCW, N_HB, 128).transpose(0, 2, 3, 1)
        .reshape(N_SC, 12, 4, 128, SCW).transpose(0, 1, 3, 2, 4)
        .reshape(N_SC, 12, 128, 4 * SCW)).astype(bf)

    in_maps = []
    for c in range(N_CORES):
        q_rows = w_qkv[c * NQ * D:(c + 1) * NQ * D]          # [768, 6144]
        k_rows = w_qkv[HID + c * D:HID + (c + 1) * D]        # [128, 6144]
        v_rows = w_qkv[HID + 8 * D + c * D:HID + 8 * D + (c + 1) * D]
        wq_c = np.concatenate([q_rows, k_rows, v_rows], axis=0)  # [1024, 6144]
        # wq[ob, p, hb, o] = wq_c[ob*128+o, hb*128+p]
        wq_arr = np.ascontiguousarray(
            wq_c.reshape(N_OB, 128, N_HB, 128).transpose(0, 3, 2, 1)).astype(bf)
        wo_c = (w_o[:, c * NQ * D:(c + 1) * NQ * D] * ATTN_MULT).T  # [768, 6144]
        # wo[mc, p, fb, m] = wo_c[fb*128+p, mc*512+m]
        wo_arr = np.ascontiguousarray(
            wo_c.reshape(N_FB, 128, N_MC, SCW).transpose(2, 1, 0, 3)).astype(bf)
        in_maps.append({
            "ht": ht, "wq": wq_arr, "wo": wo_arr,
            "cosf": cosf, "sinf": sinf, "triu": triu, "ident": ident,
            "negcap": np.full((128, 1), -SOFTCAP, np.float32),
        })
    return in_maps


_NC_CACHE = None


def _get_nc():
    global _NC_CACHE
    if _NC_CACHE is None:
        _NC_CACHE = build_nc()
    return _NC_CACHE


def kernel(positions, hidden_states, w_qkv, w_o, _trace=False, _trace_kwargs=None):
    nc = _get_nc()
    in_maps = prep_inputs(positions, hidden_states, w_qkv, w_o)
    res = run_bass_kernel_spmd(nc, in_maps, list(range(N_CORES)),
                               trace=_trace, **(_trace_kwargs or {}))
    out = np.zeros((S, HID), np.float32)
    for c in range(N_CORES):
        out += res.results[c]["out"].astype(np.float32)
    out = out.astype(np.asarray(hidden_states).dtype)
    kernel.last_results = res
    return out
